# revision 1
# baseline (speedup 1.0000x reference)
"""Fast Feedforward (FFF) tree-routing kernel for Trainium2, 8 NeuronCores.

Problem: B=8192 tokens, d=4096, binary tree depth 12 (4095 nodes).
Per token, per level: logit = <x, w1s[node]>; y += gelu(logit) * w2s[node];
node = 2*node + 1 + (logit > 0).

Strategy (data-parallel over tokens, 1024 tokens/core, 8 tiles of 128):
- Levels 0-8 (511 nodes): dense logits L = x @ W1[0:511]^T via PE matmul
  (feature-major xT chunks either host-prepped or built on-chip with PE
  transposes, see SINGLE_X). Routing = per-level select/compare on L.
  Masked gelu'd logits S (scaled by 128) combine via S^T @ W2[0:511].
- Levels 9-11: per-tile gather idx via tiny fp32 PE matmul from constant
  masks; dma_gather fetches w1 rows (bf16); per-token dot is one fused
  DVE op. w2 rows gather from an fp8 e3m4 table pre-scaled by 128
  (host-prepped; verified rel-err ~1.1% vs the 2e-2 gate) and fold into
  y via diag(gelu) fp8 matmuls. PSUM carries 128*y; the host divides the
  bf16 output by 128 (exact exponent shift).
- 3-stage software pipeline over tile triples. w2 gathers and diag
  builds are hoisted to slot start (s2_issue) and the S^T build deferred
  to slot end (st_build) so in-order SEQ queues see work in readiness
  order.
"""

import numpy as np
import ml_dtypes

import concourse.bacc as bacc
import concourse.bass as bass
import concourse.mybir as mybir
import concourse.tile as tile
from concourse.bass import ts
from concourse.masks import make_identity

P = 128
IN = 4096
OUT = 4096
DEPTH = 12
N_NODES = 2**DEPTH - 1          # 4095
N_CORES = 8
B = 8192
TOK = B // N_CORES              # 1024 tokens per core
NT = TOK // P                   # 8 tiles of 128 tokens
CH = IN // P                    # 32 feature chunks
TR = 8                          # transpose chunks per PSUM round
SH_LV = 9                       # dense shallow levels 0..8
SH_NODES = 2**SH_LV - 1         # 511
SH_PAD = 512
SH_CH = SH_PAD // P             # 4 node chunks for shallow combine
DEEP_LV = list(range(SH_LV, DEPTH))   # [9, 10, 11]
NQ = 8                          # y feature quarters
QW = OUT // NQ                  # 512
BF = mybir.dt.bfloat16
F32 = mybir.dt.float32
I16 = mybir.dt.int16
F8E3 = mybir.dt.float8e3
AF = mybir.ActivationFunctionType
OP = mybir.AluOpType

SINGLE_X = True
NSPLIT = 4                      # build xT on-chip (True) vs load from HBM
W2SCALE = 128.0                 # fp8 deep-w2 table pre-scale (exact pow2)

GELU_C0 = 0.7978845608028654    # sqrt(2/pi)
GELU_C2 = GELU_C0 * 0.044715


def emit_gelu(nc, pool, out, in_, width, tagp, dt=None, out_scale=1.0):
    """out = out_scale * gelu_tanh(in_)."""
    if dt is None:
        dt = F32
    s = pool.tile([P, width], dt, tag=tagp + "s")
    nc.vector.tensor_mul(out=s[:], in0=in_, in1=in_)
    nc.vector.tensor_scalar(out=s[:], in0=s[:], scalar1=GELU_C2,
                            scalar2=GELU_C0, op0=OP.mult, op1=OP.add)
    nc.vector.tensor_mul(out=s[:], in0=s[:], in1=in_)
    th = pool.tile([P, width], dt, tag=tagp + "t")
    nc.scalar.activation(out=th[:], in_=s[:], func=AF.Tanh)
    nc.vector.tensor_scalar(out=th[:], in0=th[:], scalar1=1.0,
                            scalar2=0.5 * out_scale, op0=OP.add, op1=OP.mult)
    nc.vector.tensor_mul(out=out, in0=th[:], in1=in_)


def build_program(n_tiles=NT, num_devices=N_CORES, dump=False,
                  skip_deep=False, skip_y=False, skip_shallow=False,
                  repeat=1):
    nc = bacc.Bacc("TRN2", target_bir_lowering=False, debug=False,
                   num_devices=num_devices, num_swdge_queues=4)
    dbg = {}
    if dump:
        for name, shape, dt in [
            ("d_ml", [P, SH_PAD], BF),
            ("d_node", [P, 1], F32), ("d_gl", [P, SH_PAD], BF),
            ("d_idx9", [P, P // 16], I16), ("d_logit9", [P, 1], F32),
            ("d_w2g9", [P, OUT], F8E3), ("d_st", [P, SH_CH, P], BF),
        ]:
            dbg[name] = nc.dram_tensor(name, shape, dt, kind="ExternalOutput")
    x_tm = nc.dram_tensor("x", [n_tiles * P, IN], BF, kind="ExternalInput")
    if not SINGLE_X:
        xT_d = nc.dram_tensor("xT", [n_tiles * P, CH * P], BF,
                              kind="ExternalInput")
    w1t_sh = nc.dram_tensor("w1t_sh", [IN, SH_PAD], BF, kind="ExternalInput")
    w1s = nc.dram_tensor("w1s", [N_NODES, IN], BF, kind="ExternalInput")
    w2sh128_d = nc.dram_tensor("w2sh128", [SH_PAD, IN], BF,
                               kind="ExternalInput")
    w2q = nc.dram_tensor("w2q", [N_NODES, IN], F8E3, kind="ExternalInput")
    y = nc.dram_tensor("y", [n_tiles * P, OUT], BF, kind="ExternalOutput")
    wsel_d = nc.dram_tensor("wsel", [P, P], F32, kind="ExternalInput")
    m8_d = nc.dram_tensor("m8", [P, 8], F32, kind="ExternalInput")

    w1t_sh_r = w1t_sh.rearrange("(c p) n -> p c n", p=P)  # [128, 32, 512]
    w2_sh_r = w2sh128_d.rearrange("(j p) f -> p j f", p=P)

    qn_counter = [0]

    def qn():
        q = qn_counter[0] % 4
        qn_counter[0] += 1
        return q

    with tile.TileContext(nc) as tc:
        with (
            tc.tile_pool(name="singles", bufs=1) as singles,
            tc.tile_pool(name="xpool", bufs=2) as xpool,
            tc.tile_pool(name="xtokpool", bufs=5) as xtokp,
            tc.tile_pool(name="spool", bufs=3) as spool,
            tc.tile_pool(name="small", bufs=16) as small,
            tc.tile_pool(name="w2gpool", bufs=10) as w2gp,
            tc.tile_pool(name="diagpool", bufs=12) as diagp,
            tc.tile_pool(name="w1gpool", bufs=2) as w1gp,
            tc.tile_pool(name="idxsave", bufs=18) as idxsave,
            tc.tile_pool(name="ypool", bufs=8) as ypool,
            tc.tile_pool(name="lps", bufs=1, space="PSUM") as lps,
            tc.tile_pool(name="idxps", bufs=1, space="PSUM") as idxps,
            tc.tile_pool(name="stps", bufs=1, space="PSUM") as stps,
            tc.tile_pool(name="yps", bufs=2, space="PSUM") as yps,
            tc.tile_pool(name="txps", bufs=3, space="PSUM") as txps,
        ):
            # --- persistent tables ---
            w1t_sb = singles.tile([P, CH, SH_PAD], BF)
            nc.scalar.dma_start(out=w1t_sb[:], in_=w1t_sh_r[:])
            w2sh_sb = singles.tile([P, SH_CH, OUT], BF)
            nc.scalar.dma_start(out=w2sh_sb[:], in_=w2_sh_r[:])
            ident = singles.tile([P, P], BF)
            make_identity(nc, ident[:])
            # wsel[i, p] = (i%16 == p%16); m8[i, cc] = (i//16 == cc).
            wsel = singles.tile([P, P], F32, tag="wsel")
            nc.gpsimd.dma_start(out=wsel[:], in_=wsel_d[:])
            m8 = singles.tile([P, 8], F32, tag="m8")
            nc.gpsimd.dma_start(out=m8[:], in_=m8_d[:])

            iota_f = singles.tile([P, SH_PAD], F32)
            nc.gpsimd.iota(iota_f[:], pattern=[[1, SH_PAD]], base=0,
                           channel_multiplier=0,
                           allow_small_or_imprecise_dtypes=True)

            state = {}

            def s0(t):
                # stage 0: x load, feature-major chunks, dense logits
                xtok = xtokp.tile([P, IN], BF, tag="xtok")
                # two half loads: shorter exclusive DMA-engine holds so the
                # latency-critical deep-chain quarter-gathers queue behind
                # at most ~1.5us of x traffic instead of ~3us
                nc.sync.dma_start(out=xtok[:, 0:IN // 2],
                                  in_=x_tm[ts(t, P), 0:IN // 2])
                nc.sync.dma_start(out=xtok[:, IN // 2:IN],
                                  in_=x_tm[ts(t, P), IN // 2:IN])
                xt = xpool.tile([P, CH, P], BF, tag="xt")
                if SINGLE_X:
                    for r in range(CH // TR):
                        tx_ps = txps.tile([P, TR, P], BF, tag="txps")
                        for j in range(TR):
                            nc.tensor.transpose(tx_ps[:, j, :],
                                                xtok[:, ts(r * TR + j, P)],
                                                ident[:])
                        nc.scalar.copy(out=xt[:, r * TR:(r + 1) * TR, :],
                                       in_=tx_ps[:])
                else:
                    nc.sync.dma_start(
                        out=xt[:],
                        in_=xT_d[ts(t, P), :].rearrange("p (c b) -> p c b",
                                                        c=CH))
                if skip_shallow:
                    state[t] = {"xtok": xtok, "l_sb": None}
                    return
                l_ps = lps.tile([P, SH_PAD], F32)
                for c in range(CH):
                    nc.tensor.matmul(l_ps[:], lhsT=xt[:, c, :],
                                     rhs=w1t_sb[:, c, :],
                                     start=(c == 0), stop=(c == CH - 1))
                l_sb = spool.tile([P, SH_PAD], F32, tag="lsb")
                nc.scalar.copy(out=l_sb[:], in_=l_ps[:])
                state[t] = {"xtok": xtok, "l_sb": l_sb}

            def s1_shallow(t):
                # shallow routing over dense logits (DVE only; gelu + S^T
                # deferred to st_build at slot end)
                stt = state[t]
                l_sb = stt["l_sb"]
                ml = spool.tile([P, SH_PAD], BF, tag="ml")
                nc.vector.memset(ml[:, SH_NODES:SH_PAD], 0.0)
                node = small.tile([P, 1], F32, tag="node")
                nc.vector.memset(node[:], 0.0)
                for d in range(0 if skip_shallow else SH_LV):
                    lo, w = 2**d - 1, 2**d
                    logit = small.tile([P, 1], F32, tag="logit")
                    nc.vector.scalar_tensor_tensor(
                        out=ml[:, lo:lo + w],
                        in0=iota_f[:, lo:lo + w],
                        scalar=node[:, :1],
                        in1=l_sb[:, lo:lo + w],
                        op0=OP.is_equal, op1=OP.mult,
                        accum_out=logit[:, :1])
                    b1 = small.tile([P, 1], F32, tag="b1")
                    nc.vector.tensor_scalar(
                        out=b1[:], in0=logit[:], scalar1=0.0, scalar2=1.0,
                        op0=OP.is_gt, op1=OP.add)
                    nc.vector.scalar_tensor_tensor(
                        out=node[:], in0=node[:], scalar=2.0, in1=b1[:],
                        op0=OP.mult, op1=OP.add)

                if dump and t == 0:
                    nc.sync.dma_start(out=dbg["d_ml"][:], in_=ml[:])
                    nc.sync.dma_start(out=dbg["d_node"][:], in_=node[:])

                stt["ml"] = ml
                stt["node"] = node
                stt["idx_t"] = {}
                stt["g_t"] = {}

            def st_build(t):
                # S = 128*gelu(ML) + S^T for next slot's combine
                stt = state[t]
                ml = stt.pop("ml")
                gl = ml
                nc.scalar.activation(out=gl[:], in_=ml[:],
                                     func=AF.Gelu_apprx_tanh)
                st_ps = stps.tile([P, SH_CH, P], BF)
                for j in range(SH_CH):
                    nc.tensor.transpose(st_ps[:, j, :], gl[:, ts(j, P)],
                                        ident[:])
                st_sb = spool.tile([P, SH_CH, P], BF, tag="stsb")
                nc.scalar.copy(out=st_sb[:], in_=st_ps[:])
                if dump and t == 0:
                    nc.sync.dma_start(out=dbg["d_gl"][:], in_=gl[:])
                    nc.sync.dma_start(out=dbg["d_st"][:], in_=st_sb[:])
                stt["st_sb"] = st_sb

            def deep_issue(t, d):
                stt = state[t]
                node = stt["node"]
                rhs8 = small.tile([P, 8], F32, tag="rhs8")
                nc.vector.tensor_scalar(out=rhs8[:], in0=m8[:],
                                        scalar1=node[:, :1],
                                        scalar2=None, op0=OP.mult)
                idx_ps = idxps.tile([P, 8], F32, tag="idxps")
                nc.tensor.matmul(idx_ps[:], lhsT=wsel[:], rhs=rhs8[:],
                                 start=True, stop=True)
                idx = idxsave.tile([P, P // 16], I16, tag="idx")
                nc.vector.tensor_copy(out=idx[:], in_=idx_ps[:])
                # split-row gather: quarter-row gathers on separate
                # queues so the first partial dot starts much earlier
                NS_ = NSPLIT
                QW_ = IN // NS_
                parts = []
                for s_ in range(NS_):
                    wp = w1gp.tile([P, 1, QW_], BF, tag=f"w1g{s_}")
                    nc.gpsimd.dma_gather(
                        wp[:], w1s[:, s_ * QW_:(s_ + 1) * QW_], idx[:, :],
                        P, P, QW_, elem_step=IN, transpose=False,
                        queue_num=qn())
                    parts.append(wp)
                if dump and t == 0 and d == SH_LV:
                    nc.sync.dma_start(out=dbg["d_idx9"][:], in_=idx[:])
                stt["idx_t"][d] = idx
                stt["w1g"] = parts

            def deep_consume(t, d):
                stt = state[t]
                node, xtok = stt["node"], stt["xtok"]
                parts = stt["w1g"]
                NS_ = NSPLIT
                QW_ = IN // NS_
                lparts = []
                for s_, wp in enumerate(parts):
                    lp_ = small.tile([P, 1], F32, tag=f"lp{s_}")
                    nc.vector.scalar_tensor_tensor(
                        out=wp[:, 0, :], in0=xtok[:, s_ * QW_:(s_ + 1) * QW_],
                        scalar=1.0, in1=wp[:, 0, :], op0=OP.bypass,
                        op1=OP.mult, accum_out=lp_[:, :1])
                    lparts.append(lp_)
                lvl = 0
                while len(lparts) > 1:
                    nxt = []
                    for k in range(0, len(lparts) - 1, 2):
                        acc = small.tile([P, 1], F32, tag=f"lr{lvl}{k}")
                        nc.vector.tensor_tensor(out=acc[:],
                                                in0=lparts[k][:],
                                                in1=lparts[k + 1][:],
                                                op=OP.add)
                        nxt.append(acc)
                    if len(lparts) % 2:
                        nxt.append(lparts[-1])
                    lparts = nxt
                    lvl += 1
                logit = lparts[0]
                if dump and t == 0 and d == SH_LV:
                    nc.sync.dma_start(out=dbg["d_logit9"][:], in_=logit[:])
                g_bf = idxsave.tile([P, 1], F32, tag="gbf")
                nc.scalar.activation(out=g_bf[:], in_=logit[:],
                                     func=AF.Gelu_apprx_tanh)
                stt["g_t"][d] = g_bf
                if d < DEPTH - 1:
                    b1 = small.tile([P, 1], F32, tag="b1")
                    nc.vector.tensor_scalar(
                        out=b1[:], in0=logit[:], scalar1=0.0,
                        scalar2=1.0, op0=OP.is_gt, op1=OP.add)
                    nc.vector.scalar_tensor_tensor(
                        out=node[:], in0=node[:], scalar=2.0, in1=b1[:],
                        op0=OP.mult, op1=OP.add)

            def s2_issue(t):
                # slot start: fp8 w2 gathers + diag build (idx/g ready)
                stt = state[t]
                idx_t, g_t = stt["idx_t"], stt["g_t"]
                deep_lv = [] if skip_deep else DEEP_LV
                stt["diag_t"] = {}
                stt["w2g_t"] = {}
                for d in deep_lv:
                    dg = diagp.tile([P, P], F8E3, tag="diag")
                    nc.gpsimd.tensor_scalar(
                        out=dg[:], in0=ident[:], scalar1=g_t[d][:, :1],
                        scalar2=None, op0=OP.mult)
                    stt["diag_t"][d] = dg
                    w2g = w2gp.tile([P, 1, IN], F8E3, tag="w2g")
                    nc.gpsimd.dma_gather(
                        w2g[:], w2q[:, :], idx_t[d][:], P, P, IN,
                        transpose=False, queue_num=qn())
                    stt["w2g_t"][d] = w2g
                    if dump and t == 0 and d == SH_LV:
                        nc.sync.dma_start(out=dbg["d_w2g9"][:],
                                          in_=w2g[:, 0, :])

            def s2(t):
                # y combine + store
                stt = state.pop(t)
                st_sb = stt["st_sb"]
                diag_t, w2g_t = stt["diag_t"], stt["w2g_t"]
                deep_lv = [] if skip_deep else DEEP_LV
                if skip_y:
                    y_sb = ypool.tile([P, QW], BF, tag="ysb")
                    nc.vector.memset(y_sb[:], 0.0)
                    for q in range(NQ):
                        nc.sync.dma_start(out=y[ts(t, P), ts(q, QW)],
                                          in_=y_sb[:])
                    return
                for q in range(NQ):
                    y_ps = yps.tile([P, QW], F32)
                    col0 = q * QW
                    first = True
                    for d in deep_lv:
                        nc.tensor.matmul(
                            y_ps[:], lhsT=diag_t[d][:],
                            rhs=w2g_t[d][:, 0, col0:col0 + QW],
                            start=first, stop=False)
                        first = False
                    for j in range(SH_CH):
                        nc.tensor.matmul(
                            y_ps[:], lhsT=st_sb[:, j, :],
                            rhs=w2sh_sb[:, j, col0:col0 + QW],
                            start=first, stop=(j == SH_CH - 1))
                        first = False
                    y_sb = ypool.tile([P, QW], BF, tag="ysb")
                    nc.scalar.activation(out=y_sb[:], in_=y_ps[:],
                                         func=AF.Identity,
                                         scale=1.0 / W2SCALE)
                    nc.sync.dma_start(out=y[ts(t, P), ts(q, QW)],
                                      in_=y_sb[:])

            deep_lv = [] if skip_deep else DEEP_LV
            cuts = [0, 3, 6, n_tiles]
            groups = [list(range(cuts[i], cuts[i + 1]))
                      for i in range(len(cuts) - 1)]
            ng = len(groups)

            def s1_group(tiles):
                if not deep_lv:
                    for a in tiles:
                        s1_shallow(a)
                        st_build(a)
                    return
                for a in tiles:
                    s1_shallow(a)
                    deep_issue(a, deep_lv[0])
                for d in deep_lv:
                    for a in tiles:
                        deep_consume(a, d)
                        if d + 1 in deep_lv:
                            deep_issue(a, d + 1)

            def emit_slot(m):
                if m >= 2:
                    for a in groups[m - 2]:
                        s2_issue(a)
                if m < ng:
                    for a in groups[m]:
                        s0(a)
                if 1 <= m <= ng:
                    s1_group(groups[m - 1])
                if m >= 2:
                    for a in groups[m - 2]:
                        s2(a)
                if 1 <= m <= ng and deep_lv:
                    for a in groups[m - 1]:
                        st_build(a)

            for _rep in range(repeat):
                for m in range(ng + 2):
                    emit_slot(m)

    nc.compile()
    return nc


_CACHED = {}


def _get_program(n_tiles=NT, num_devices=N_CORES):
    key = (n_tiles, num_devices)
    if key not in _CACHED:
        _CACHED[key] = build_program(n_tiles, num_devices)
    return _CACHED[key]


def idx_masks():
    i = np.arange(P)
    wsel = (i[:, None] % 16 == i[None, :] % 16).astype(np.float32)
    m8 = (i[:, None] // 16 == np.arange(8)[None, :]).astype(np.float32)
    return wsel, m8


def prep_inputs(input, w1s, w2s):
    """Host-side layout prep shared by all cores."""
    w1 = np.asarray(w1s)
    w1t_sh = np.zeros((IN, SH_PAD), dtype=w1.dtype)
    w1t_sh[:, :SH_NODES] = w1[:SH_NODES].T
    w2q = (np.asarray(w2s).astype(np.float32) * W2SCALE).astype(
        ml_dtypes.float8_e3m4)
    w2sh128 = (np.asarray(w2s)[0:SH_PAD].astype(np.float32)
               * W2SCALE).astype(ml_dtypes.bfloat16)
    return np.ascontiguousarray(w1t_sh), w2q, np.ascontiguousarray(w2sh128)


def prep_xT(input):
    x = np.asarray(input)
    xr = x.reshape(B // P, P, CH, P).transpose(0, 3, 2, 1)
    return np.ascontiguousarray(xr.reshape(B, CH * P))


def _run(input, w1s, w2s, **spmd_kwargs):
    from concourse.bass_utils import run_bass_kernel_spmd

    nc = _get_program()
    w1t_sh, w2q, w2sh128 = prep_inputs(input, w1s, w2s)
    w1 = np.ascontiguousarray(np.asarray(w1s))
    wsel, m8 = idx_masks()
    xT = None if SINGLE_X else prep_xT(input)
    in_maps = []
    for c in range(N_CORES):
        im = {
            "x": np.ascontiguousarray(np.asarray(input)[c * TOK:(c + 1) * TOK]),
            "w1t_sh": w1t_sh,
            "w1s": w1,
            "w2sh128": w2sh128,
            "w2q": w2q,
            "wsel": wsel,
            "m8": m8,
        }
        if not SINGLE_X:
            im["xT"] = np.ascontiguousarray(xT[c * TOK:(c + 1) * TOK, :])
        in_maps.append(im)
    res = run_bass_kernel_spmd(nc, in_maps, core_ids=list(range(N_CORES)),
                               **spmd_kwargs)
    # device computes 128*y (fp8 table pre-scale); exact pow2 descale here
    out = np.concatenate([res.results[c]["y"] for c in range(N_CORES)],
                         axis=0)
    return out.astype(ml_dtypes.bfloat16), res


def kernel(input, w1s, w2s, depth):
    assert int(depth) == DEPTH
    out, _ = _run(input, w1s, w2s)
    return out



# revision 49
# speedup vs baseline: 1.0857x; 1.0857x over previous
"""Fast Feedforward (FFF) tree-routing kernel for Trainium2, 8 NeuronCores.

Problem: B=8192 tokens, d=4096, binary tree depth 12 (4095 nodes).
Per token, per level: logit = <x, w1s[node]>; y += gelu(logit) * w2s[node];
node = 2*node + 1 + (logit > 0).

Strategy (data-parallel over tokens, 1024 tokens/core, 8 tiles of 128):
- Levels 0-8 (511 nodes): dense logits L = x @ W1[0:511]^T via PE matmul
  (xT built on-chip with PE transposes). Routing = per-level select/compare
  on L (DVE). Masked gelu'd logits S^T combine against an fp8 e3m4 shallow
  w2 table (sliced from the full w2q table, pre-scaled by 128).
- Levels 9-11: per-tile full-row dma_gathers; levels 9-10 gather w1 in
  bf16 (routing-precision-critical), level 11 in fp8 e3m4 x128 (value-only;
  sign flips at the last level are harmless). Per-token dot is one fused
  DVE op; y folds in via diag(gelu) fp8 matmuls. PSUM carries 128*y; the
  final copy divides by 128 (exact exponent shift).
- 4-group software pipeline (2 tiles/group): slot m runs combine(m-2),
  deep ladder(m-1) with w2 gathers issued level-by-level, transposes+
  logits(m), x prefetch(m+1). Combine tiles are emitted between ladder
  levels so the PE queue always has ready work while routing latency
  resolves.
"""

import numpy as np
import ml_dtypes

import concourse.bacc as bacc
import concourse.bass as bass
import concourse.mybir as mybir
import concourse.tile as tile
from concourse.bass import ts
from concourse.masks import make_identity

P = 128
IN = 4096
OUT = 4096
DEPTH = 12
N_NODES = 2**DEPTH - 1          # 4095
N_CORES = 8
B = 8192
TOK = B // N_CORES              # 1024 tokens per core
NT = TOK // P                   # 8 tiles of 128 tokens
CH = IN // P                    # 32 feature chunks
TR = 8                          # transpose chunks per PSUM round
SH_LV = 9                       # dense shallow levels 0..8
SH_NODES = 2**SH_LV - 1         # 511
SH_PAD = 512
SH_CH = SH_PAD // P             # 4 node chunks for shallow combine
DEEP_LV = list(range(SH_LV, DEPTH))   # [9, 10, 11]
LAST_LV = DEPTH - 1
NQ = 8                          # y feature quarters
QW = OUT // NQ                  # 512
BF = mybir.dt.bfloat16
F32 = mybir.dt.float32
I16 = mybir.dt.int16
F8E3 = mybir.dt.float8e3
AF = mybir.ActivationFunctionType
OP = mybir.AluOpType

W2SCALE = 128.0                 # fp8 w2 table pre-scale (exact pow2)
W1SCALE = 128.0                 # fp8 deep-w1 table pre-scale (exact pow2)
LVL11_FP8 = False               # gather last-level w1 rows as fp8 e3m4
CUTS = [0, 2, 4, 6, NT]         # pipeline groups

# emission-order tuning knobs (swept during development)
OPTS = {
    "early9": False,        # launch level-9 gathers at end of prior slot
    "idxcopy_pool": False,  # idx PSUM->SBUF copy on gpsimd vs DVE
    "idxps2": False,        # 2 idx PSUM banks (st_build then shares txps)
    "comb_chunks": ((0, 2), (2, 5), (5, 8)),
}


def build_program(n_tiles=NT, num_devices=N_CORES):
    nc = bacc.Bacc("TRN2", target_bir_lowering=False, debug=False,
                   num_devices=num_devices, num_swdge_queues=4)
    x_tm = nc.dram_tensor("x", [n_tiles * P, IN], BF, kind="ExternalInput")
    w1t_sh = nc.dram_tensor("w1t_sh", [IN, SH_PAD], BF, kind="ExternalInput")
    w1s = nc.dram_tensor("w1s", [N_NODES, IN], BF, kind="ExternalInput")
    w1q8 = nc.dram_tensor("w1q8", [N_NODES, IN], F8E3, kind="ExternalInput")
    w2q = nc.dram_tensor("w2q", [N_NODES, IN], F8E3, kind="ExternalInput")
    y = nc.dram_tensor("y", [n_tiles * P, OUT], BF, kind="ExternalOutput")
    wsel_d = nc.dram_tensor("wsel", [P, P], F32, kind="ExternalInput")
    m8_d = nc.dram_tensor("m8", [P, 8], F32, kind="ExternalInput")

    w1t_sh_r = w1t_sh.rearrange("(c p) n -> p c n", p=P)   # [128, 32, 512]
    w2_sh_r = w2q[0:SH_PAD, :].rearrange("(j p) f -> p j f", p=P)

    qn_counter = [0]

    def qn():
        q = qn_counter[0] % 4
        qn_counter[0] += 1
        return q

    groups = [list(range(CUTS[i], CUTS[i + 1])) for i in range(len(CUTS) - 1)]
    ng = len(groups)

    with tile.TileContext(nc) as tc:
        with (
            tc.tile_pool(name="singles", bufs=1) as singles,
            tc.tile_pool(name="xpool", bufs=2) as xpool,
            tc.tile_pool(name="xtokpool", bufs=5) as xtokp,
            tc.tile_pool(name="spool", bufs=3) as spool,
            tc.tile_pool(name="small", bufs=16) as small,
            tc.tile_pool(name="w2gpool", bufs=9) as w2gp,
            tc.tile_pool(name="diagpool", bufs=12) as diagp,
            tc.tile_pool(name="w1gpool", bufs=5) as w1gp,
            tc.tile_pool(name="w1g8pool", bufs=2) as w1g8p,
            tc.tile_pool(name="gsave", bufs=12) as gsave,
            tc.tile_pool(name="ypool", bufs=8) as ypool,
            tc.tile_pool(name="lps", bufs=2, space="PSUM") as lps,
            tc.tile_pool(name="idxps", bufs=2 if OPTS["idxps2"] else 1,
                         space="PSUM") as idxps,
            tc.tile_pool(name="stps", bufs=1, space="PSUM") as stps,
            tc.tile_pool(name="yps", bufs=2, space="PSUM") as yps,
            tc.tile_pool(name="txps", bufs=2, space="PSUM") as txps,
        ):
            # --- persistent tables (split loads so the DMA engine hands
            # pieces to the first tiles' compute as they arrive) ---
            ident = singles.tile([P, P], BF)
            make_identity(nc, ident[:])
            wsel = singles.tile([P, P], F32, tag="wsel")
            nc.gpsimd.dma_start(out=wsel[:], in_=wsel_d[:])
            m8 = singles.tile([P, 8], F32, tag="m8")
            nc.gpsimd.dma_start(out=m8[:], in_=m8_d[:])
            iota_f = singles.tile([P, SH_PAD], F32)
            nc.gpsimd.iota(iota_f[:], pattern=[[1, SH_PAD]], base=0,
                           channel_multiplier=0,
                           allow_small_or_imprecise_dtypes=True)
            w1t_sb = singles.tile([P, CH, SH_PAD], BF)
            w2sh_sb = singles.tile([P, SH_CH, OUT], F8E3)

            state = {}

            def load_w1t_chunk(r):
                nc.sync.dma_start(
                    out=w1t_sb[:, r * 8:(r + 1) * 8, :],
                    in_=w1t_sh_r[:, r * 8:(r + 1) * 8, :])

            def load_w2sh_chunk(r):
                nc.sync.dma_start(
                    out=w2sh_sb[:, r * 2:(r + 1) * 2, :],
                    in_=w2_sh_r[:, r * 2:(r + 1) * 2, :])

            def s0_load(t):
                xtok = xtokp.tile([P, IN], BF, tag="xtok")
                nc.sync.dma_start(out=xtok[:, 0:IN // 2],
                                  in_=x_tm[ts(t, P), 0:IN // 2])
                nc.sync.dma_start(out=xtok[:, IN // 2:IN],
                                  in_=x_tm[ts(t, P), IN // 2:IN])
                state[t] = {"xtok": xtok}

            def s0_transpose(t):
                stt = state[t]
                xtok = stt["xtok"]
                xt = xpool.tile([P, CH, P], BF, tag="xt")
                for r in range(CH // TR):
                    tx_ps = txps.tile([P, TR, P], BF, tag="txps")
                    for j in range(TR):
                        nc.tensor.transpose(tx_ps[:, j, :],
                                            xtok[:, ts(r * TR + j, P)],
                                            ident[:])
                    nc.scalar.copy(out=xt[:, r * TR:(r + 1) * TR, :],
                                   in_=tx_ps[:])
                stt["xt"] = xt

            def s0_logits(t):
                stt = state[t]
                xt = stt.pop("xt")
                l_ps = lps.tile([P, SH_PAD], F32)
                for c in range(CH):
                    nc.tensor.matmul(l_ps[:], lhsT=xt[:, c, :],
                                     rhs=w1t_sb[:, c, :],
                                     start=(c == 0), stop=(c == CH - 1))
                l_sb = spool.tile([P, SH_PAD], F32, tag="lsb")
                nc.vector.tensor_copy(out=l_sb[:], in_=l_ps[:])
                stt["l_sb"] = l_sb

            def s1_shallow(t):
                # shallow routing over dense logits (DVE only; gelu + S^T
                # deferred to st_build at slot end)
                stt = state[t]
                l_sb = stt["l_sb"]
                ml = spool.tile([P, SH_PAD], BF, tag="ml")
                nc.vector.memset(ml[:, SH_NODES:SH_PAD], 0.0)
                node = small.tile([P, 1], F32, tag="node")
                nc.vector.memset(node[:], 0.0)
                for d in range(SH_LV):
                    lo, w = 2**d - 1, 2**d
                    logit = small.tile([P, 1], F32, tag="logit")
                    nc.vector.scalar_tensor_tensor(
                        out=ml[:, lo:lo + w],
                        in0=iota_f[:, lo:lo + w],
                        scalar=node[:, :1],
                        in1=l_sb[:, lo:lo + w],
                        op0=OP.is_equal, op1=OP.mult,
                        accum_out=logit[:, :1])
                    b1 = small.tile([P, 1], F32, tag="b1")
                    nc.vector.tensor_scalar(
                        out=b1[:], in0=logit[:], scalar1=0.0, scalar2=1.0,
                        op0=OP.is_gt, op1=OP.add)
                    nc.vector.scalar_tensor_tensor(
                        out=node[:], in0=node[:], scalar=2.0, in1=b1[:],
                        op0=OP.mult, op1=OP.add)
                stt["ml"] = ml
                stt["node"] = node
                stt["diag_t"] = {}
                stt["w2g_t"] = {}

            def st_build(t):
                # S = gelu(ML); S^T chunks for next slot's combine
                stt = state[t]
                ml = stt.pop("ml")
                gl = ml
                nc.scalar.activation(out=gl[:], in_=ml[:],
                                     func=AF.Gelu_apprx_tanh)
                if OPTS["idxps2"]:
                    st_ps = txps.tile([P, TR, P], BF, tag="txps")
                else:
                    st_ps = stps.tile([P, SH_CH, P], BF, tag="st_ps")
                for j in range(SH_CH):
                    nc.tensor.transpose(st_ps[:, j, :], gl[:, ts(j, P)],
                                        ident[:])
                st_sb = spool.tile([P, SH_CH, P], BF, tag="stsb")
                nc.scalar.copy(out=st_sb[:], in_=st_ps[:, 0:SH_CH, :])
                stt["st_sb"] = st_sb

            def deep_issue(t, d):
                # idx from node via PE select trick, then one full-row w1
                # gather (bf16 for 9/10, fp8 for the last level) and the
                # level's w2 fp8 gather immediately behind it.
                stt = state[t]
                node = stt["node"]
                rhs8 = small.tile([P, 8], F32, tag="rhs8")
                nc.vector.tensor_scalar(out=rhs8[:], in0=m8[:],
                                        scalar1=node[:, :1],
                                        scalar2=None, op0=OP.mult)
                idx_ps = idxps.tile([P, 8], F32, tag="idxps")
                nc.tensor.matmul(idx_ps[:], lhsT=wsel[:], rhs=rhs8[:],
                                 start=True, stop=True)
                # idx copy: gpsimd variant lands directly before the gather
                # in the same Pool queue (no DVE round-trip)
                idx = small.tile([P, P // 16], I16, tag="idx")
                if OPTS["idxcopy_pool"]:
                    nc.gpsimd.tensor_copy(out=idx[:], in_=idx_ps[:])
                else:
                    nc.vector.tensor_copy(out=idx[:], in_=idx_ps[:])
                if d < LAST_LV or not LVL11_FP8:
                    wp = w1gp.tile([P, 1, IN], BF, tag="w1g")
                    nc.gpsimd.dma_gather(
                        wp[:], w1s[:, :], idx[:, :], P, P, IN,
                        transpose=False, queue_num=qn())
                else:
                    wp = w1g8p.tile([P, 1, IN], F8E3, tag="w1g8")
                    nc.gpsimd.dma_gather(
                        wp[:], w1q8[:, :], idx[:, :], P, P, IN,
                        transpose=False, queue_num=qn())
                stt["w1g"] = wp
                if d == SH_LV and OPTS["early9"]:
                    # w2 row gather for level 9 is deferred to next slot's
                    # start (not chain-critical; keeps its buffer lifetime
                    # short when level-9 w1 gathers launch a slot early)
                    stt["idx9"] = idx
                else:
                    issue_w2(t, d, idx)

            def issue_w2(t, d, idx):
                stt = state[t]
                w2g = w2gp.tile([P, 1, IN], F8E3, tag="w2g")
                nc.gpsimd.dma_gather(
                    w2g[:], w2q[:, :], idx[:], P, P, IN,
                    transpose=False, queue_num=qn())
                stt["w2g_t"][d] = w2g

            def deep_consume(t, d):
                stt = state[t]
                node, xtok = stt["node"], stt["xtok"]
                wp = stt.pop("w1g")
                col = d - SH_LV
                if col == 0:
                    lg = gsave.tile([P, 4], F32, tag="lg")
                    stt["lg"] = lg
                lg = stt["lg"]
                # levels 9/10 route on the logit sign, so they use the
                # fused scalar_tensor_tensor dot (slow 1x DVE mode but the
                # accumulator matches the reference's precision; summing
                # bf16-rounded products flips ~6 routing bits per level).
                # The last level is value-only, so it takes the fast path:
                # 2x-mode multiply + 4x-mode accumulate over bf16 products.
                if d < LAST_LV:
                    nc.vector.scalar_tensor_tensor(
                        out=wp[:, 0, :], in0=xtok[:, :],
                        scalar=1.0, in1=wp[:, 0, :], op0=OP.bypass,
                        op1=OP.mult, accum_out=lg[:, col:col + 1])
                else:
                    if LVL11_FP8:
                        # fp8 w1 row: products to a bf16 scratch (reuse
                        # the w1g ring); accumulates 128*<x,w1>.
                        prod = w1gp.tile([P, 1, IN], BF, tag="w1g")
                        prod_ap = prod[:, 0, :]
                        sc = 1.0 / W1SCALE
                    else:
                        prod_ap = wp[:, 0, :]
                        sc = 1.0
                    nc.vector.tensor_tensor(
                        out=prod_ap, in0=xtok[:, :], in1=wp[:, 0, :],
                        op=OP.mult)
                    nc.vector.tensor_scalar(
                        out=prod_ap, in0=prod_ap, scalar1=sc, scalar2=0.0,
                        op0=OP.mult, op1=OP.add,
                        accum_out=lg[:, col:col + 1])
                if d < LAST_LV:
                    b1 = small.tile([P, 1], F32, tag="b1")
                    nc.vector.tensor_scalar(
                        out=b1[:], in0=lg[:, col:col + 1], scalar1=0.0,
                        scalar2=1.0, op0=OP.is_gt, op1=OP.add)
                    nc.vector.scalar_tensor_tensor(
                        out=node[:], in0=node[:], scalar=2.0, in1=b1[:],
                        op0=OP.mult, op1=OP.add)

            def deep_finish(t):
                # one batched gelu over the 3 deep logits, then the three
                # fp8 diag builds for next slot's combine
                stt = state[t]
                lg = stt.pop("lg")
                g3 = gsave.tile([P, 4], F32, tag="g3")
                nc.scalar.activation(out=g3[:, 0:3], in_=lg[:, 0:3],
                                     func=AF.Gelu_apprx_tanh)
                for d in DEEP_LV:
                    dg = diagp.tile([P, P], F8E3, tag="diag")
                    nc.gpsimd.tensor_scalar(
                        out=dg[:], in0=ident[:],
                        scalar1=g3[:, d - SH_LV:d - SH_LV + 1],
                        scalar2=None, op0=OP.mult)
                    stt["diag_t"][d] = dg

            def s2(t, qlo=0, qhi=NQ):
                # y combine + store: per quarter, 3 deep diag fp8 matmuls +
                # 4 shallow S^T chunks against the fp8 shallow table.
                stt = state[t]
                st_sb = stt["st_sb"]
                diag_t, w2g_t = stt["diag_t"], stt["w2g_t"]
                for q in range(qlo, qhi):
                    y_ps = yps.tile([P, QW], F32)
                    col0 = q * QW
                    for j in range(SH_CH):
                        nc.tensor.matmul(
                            y_ps[:], lhsT=st_sb[:, j, :],
                            rhs=w2sh_sb[:, j, col0:col0 + QW],
                            start=(j == 0), stop=False)
                    for d in DEEP_LV:
                        nc.tensor.matmul(
                            y_ps[:], lhsT=diag_t[d][:],
                            rhs=w2g_t[d][:, 0, col0:col0 + QW],
                            start=False, stop=(d == LAST_LV))
                    y_sb = ypool.tile([P, QW], BF, tag="ysb")
                    nc.scalar.activation(out=y_sb[:], in_=y_ps[:],
                                         func=AF.Identity,
                                         scale=1.0 / W2SCALE)
                    nc.sync.dma_start(out=y[ts(t, P), ts(q, QW)],
                                      in_=y_sb[:])

            def emit_slot(m):
                gm1 = groups[m - 1] if 1 <= m <= ng else []
                gm2 = groups[m - 2] if 2 <= m <= ng + 1 else []
                gm0 = groups[m] if m < ng else []
                gp1 = groups[m + 1] if m + 1 < ng else []

                # combine work of group m-2, emitted in chunks as PE cover:
                # each batch lands BEFORE the next ladder level's idx
                # matmuls, so the PE queue head is ready work while the
                # previous level's gather+dot latency resolves
                if OPTS["early9"]:
                    comb = [(t, q, q + 1) for t in gm2 for q in range(NQ)]
                else:
                    comb = [(t, ql, qh) for t in gm2
                            for (ql, qh) in OPTS["comb_chunks"]]

                def emit_comb(n):
                    for _ in range(n):
                        if comb:
                            s2(*comb.pop(0))

                if OPTS["early9"]:
                    # group m-1's level-9 gathers were issued at the END of
                    # the previous slot (right after its routing), so this
                    # slot's ladder is only the level-10/11 chain.
                    for t in gm1:
                        issue_w2(t, SH_LV, state[t].pop("idx9"))
                    for t in gm1:
                        st_build(t)
                    QUS = 1.5              # PE us per combine quarter
                    LAT = 11.0             # per-level ladder chain latency
                    STAG = 3.0             # per-tile dot stagger on DVE
                    sched = []
                    for lv_i, d in enumerate(DEEP_LV[1:], start=1):
                        for t_i, t in enumerate(gm1):
                            sched.append(
                                (4.0 + (lv_i - 1) * LAT + t_i * STAG,
                                 d, t_i, t))
                    sched.sort()
                    pos = 0.0
                    for when, d, t_i, t in sched:
                        want = int(max(0.0, when - pos) / QUS + 0.5)
                        nq_emit = min(want, len(comb))
                        emit_comb(nq_emit)
                        pos += nq_emit * QUS
                        deep_consume(t, d - 1)
                        deep_issue(t, d)
                    emit_comb(len(comb))
                    for t in gm1:
                        deep_consume(t, LAST_LV)
                    for t in gm1:
                        deep_finish(t)
                else:
                    # classic: ladder fully inside this slot, level-major,
                    # combine chunks of group m-2 between levels
                    emit_comb(1)
                    for t in gm1:
                        deep_issue(t, SH_LV)
                    nlv = len(DEEP_LV)
                    for lv_i, d in enumerate(DEEP_LV):
                        if d > SH_LV:
                            left = nlv - lv_i + 1
                            emit_comb(max(1, len(comb) // left))
                            for t in gm1:
                                deep_issue(t, d)
                        for t in gm1:
                            deep_consume(t, d)
                    emit_comb(len(comb))
                    for t in gm1:
                        deep_finish(t)
                    for t in gm1:
                        st_build(t)
                for t in gm2:
                    state.pop(t, None)
                # next group's transposes + logits + its shallow routing,
                # so the next slot's ladder starts without routing latency
                for t in gm0:
                    s0_transpose(t)
                    s0_logits(t)
                for t in gm0:
                    s1_shallow(t)
                if OPTS["early9"]:
                    for t in gm0:
                        deep_issue(t, SH_LV)
                # x prefetch for group m+1
                for t in gp1:
                    s0_load(t)

            # prologue: x for group 0 interleaved with the shallow tables
            s0_load(groups[0][0])
            load_w1t_chunk(0)
            for t in groups[0][1:]:
                s0_load(t)
            for r in range(1, 4):
                load_w1t_chunk(r)
            for r in range(2):
                load_w2sh_chunk(r)
            for m in range(ng + 2):
                emit_slot(m)

    nc.compile()
    return nc


_CACHED = {}


def _get_program(n_tiles=NT, num_devices=N_CORES):
    key = (n_tiles, num_devices)
    if key not in _CACHED:
        _CACHED[key] = build_program(n_tiles, num_devices)
    return _CACHED[key]


def idx_masks():
    i = np.arange(P)
    wsel = (i[:, None] % 16 == i[None, :] % 16).astype(np.float32)
    m8 = (i[:, None] // 16 == np.arange(8)[None, :]).astype(np.float32)
    return wsel, m8


def prep_inputs(input, w1s, w2s):
    """Host-side layout prep shared by all cores."""
    w1 = np.asarray(w1s)
    w1t_sh = np.zeros((IN, SH_PAD), dtype=w1.dtype)
    w1t_sh[:, :SH_NODES] = w1[:SH_NODES].T
    w1q8 = (w1.astype(np.float32) * W1SCALE).astype(ml_dtypes.float8_e3m4)
    w2q = (np.asarray(w2s).astype(np.float32) * W2SCALE).astype(
        ml_dtypes.float8_e3m4)
    return np.ascontiguousarray(w1t_sh), w1q8, w2q


def _run(input, w1s, w2s, **spmd_kwargs):
    from concourse.bass_utils import run_bass_kernel_spmd

    nc = _get_program()
    w1t_sh, w1q8, w2q = prep_inputs(input, w1s, w2s)
    w1 = np.ascontiguousarray(np.asarray(w1s))
    wsel, m8 = idx_masks()
    in_maps = []
    for c in range(N_CORES):
        im = {
            "x": np.ascontiguousarray(np.asarray(input)[c * TOK:(c + 1) * TOK]),
            "w1t_sh": w1t_sh,
            "w1s": w1,
            "w1q8": w1q8,
            "w2q": w2q,
            "wsel": wsel,
            "m8": m8,
        }
        in_maps.append(im)
    res = run_bass_kernel_spmd(nc, in_maps, core_ids=list(range(N_CORES)),
                               **spmd_kwargs)
    out = np.concatenate([res.results[c]["y"] for c in range(N_CORES)],
                         axis=0)
    return out.astype(ml_dtypes.bfloat16), res


def kernel(input, w1s, w2s, depth):
    assert int(depth) == DEPTH
    out, _ = _run(input, w1s, w2s)
    return out


# revision 61
# speedup vs baseline: 1.0864x; 1.0007x over previous
"""Fast Feedforward (FFF) tree-routing kernel for Trainium2, 8 NeuronCores.

Problem: B=8192 tokens, d=4096, binary tree depth 12 (4095 nodes).
Per token, per level: logit = <x, w1s[node]>; y += gelu(logit) * w2s[node];
node = 2*node + 1 + (logit > 0).

Strategy (data-parallel over tokens, 1024 tokens/core, 8 tiles of 128):
- Levels 0-8 (511 nodes): dense logits L = x @ W1[0:511]^T via PE matmul
  (xT built on-chip with PE transposes). Routing = per-level select/compare
  on L (DVE). Masked gelu'd logits S^T combine against an fp8 e3m4 shallow
  w2 table (sliced from the full w2q table, pre-scaled by 128).
- Levels 9-11: per-tile full-row w1/w2 dma_gathers (w1 bf16, w2 fp8).
  Levels 9/10 dot via the fused scalar_tensor_tensor (slower 1x DVE mode
  but reference-matching fp32 accumulation - summing bf16-rounded
  products flips routing bits); level 11 is value-only so it uses the
  fast 2x-multiply + 4x-accumulate pair. One batched gelu per tile at
  ladder end; y folds in via diag(gelu) fp8 matmuls. PSUM carries 128*y;
  the final copy divides by 128 (exact exponent shift).
- 4-group software pipeline (2 tiles/group): slot m runs combine(m-2)
  split into chunks emitted between ladder levels of group m-1 (so the
  PE queue head is always ready work while gather+dot latency resolves),
  then transposes+logits+routing(m) and x prefetch(m+1). Table loads are
  split into ~1MB pieces interleaved with the first x tiles so the first
  logits matmuls chase the arriving w1t chunks.
"""

import numpy as np
import ml_dtypes

import concourse.bacc as bacc
import concourse.bass as bass
import concourse.mybir as mybir
import concourse.tile as tile
from concourse.bass import ts
from concourse.masks import make_identity

P = 128
IN = 4096
OUT = 4096
DEPTH = 12
N_NODES = 2**DEPTH - 1          # 4095
N_CORES = 8
B = 8192
TOK = B // N_CORES              # 1024 tokens per core
NT = TOK // P                   # 8 tiles of 128 tokens
CH = IN // P                    # 32 feature chunks
TR = 8                          # transpose chunks per PSUM round
SH_LV = 9                       # dense shallow levels 0..8
SH_NODES = 2**SH_LV - 1         # 511
SH_PAD = 512
SH_CH = SH_PAD // P             # 4 node chunks for shallow combine
DEEP_LV = list(range(SH_LV, DEPTH))   # [9, 10, 11]
LAST_LV = DEPTH - 1
NQ = 8                          # y feature quarters
QW = OUT // NQ                  # 512
BF = mybir.dt.bfloat16
F32 = mybir.dt.float32
I16 = mybir.dt.int16
F8E3 = mybir.dt.float8e3
AF = mybir.ActivationFunctionType
OP = mybir.AluOpType

W2SCALE = 128.0                 # fp8 w2 table pre-scale (exact pow2)
W1SCALE = 128.0                 # fp8 deep-w1 table pre-scale (exact pow2)
LVL11_FP8 = False               # gather last-level w1 rows as fp8 e3m4
CUTS = [0, 2, 4, 6, NT]         # pipeline groups

# emission-order tuning knobs (swept during development)
OPTS = {
    "early9": False,        # launch level-9 gathers at end of prior slot
    "idxcopy_pool": False,  # idx PSUM->SBUF copy on gpsimd vs DVE
    "idxps2": False,        # 2 idx PSUM banks (st_build then shares txps)
    "comb_chunks": ((0, 2), (2, 5), (5, 8)),
    "x_pieces": 4,          # x tile loaded in 4 slices (less DMA blocking)
}


def build_program(n_tiles=NT, num_devices=N_CORES):
    nc = bacc.Bacc("TRN2", target_bir_lowering=False, debug=False,
                   num_devices=num_devices, num_swdge_queues=4)
    x_tm = nc.dram_tensor("x", [n_tiles * P, IN], BF, kind="ExternalInput")
    w1t_sh = nc.dram_tensor("w1t_sh", [IN, SH_PAD], BF, kind="ExternalInput")
    w1s = nc.dram_tensor("w1s", [N_NODES, IN], BF, kind="ExternalInput")
    w1q8 = nc.dram_tensor("w1q8", [N_NODES, IN], F8E3, kind="ExternalInput")
    w2q = nc.dram_tensor("w2q", [N_NODES, IN], F8E3, kind="ExternalInput")
    y = nc.dram_tensor("y", [n_tiles * P, OUT], BF, kind="ExternalOutput")
    wsel_d = nc.dram_tensor("wsel", [P, P], F32, kind="ExternalInput")
    m8_d = nc.dram_tensor("m8", [P, 8], F32, kind="ExternalInput")

    w1t_sh_r = w1t_sh.rearrange("(c p) n -> p c n", p=P)   # [128, 32, 512]
    w2_sh_r = w2q[0:SH_PAD, :].rearrange("(j p) f -> p j f", p=P)

    qn_counter = [0]

    def qn():
        q = qn_counter[0] % 4
        qn_counter[0] += 1
        return q

    groups = [list(range(CUTS[i], CUTS[i + 1])) for i in range(len(CUTS) - 1)]
    ng = len(groups)

    with tile.TileContext(nc) as tc:
        with (
            tc.tile_pool(name="singles", bufs=1) as singles,
            tc.tile_pool(name="xpool", bufs=2) as xpool,
            tc.tile_pool(name="xtokpool", bufs=OPTS.get("xtok_bufs", 5)) as xtokp,
            tc.tile_pool(name="spool", bufs=3) as spool,
            tc.tile_pool(name="small", bufs=16) as small,
            tc.tile_pool(name="w2gpool", bufs=9) as w2gp,
            tc.tile_pool(name="diagpool", bufs=12) as diagp,
            tc.tile_pool(name="w1gpool", bufs=5) as w1gp,
            tc.tile_pool(name="w1g8pool", bufs=2) as w1g8p,
            tc.tile_pool(name="gsave", bufs=12) as gsave,
            tc.tile_pool(name="ypool", bufs=8) as ypool,
            tc.tile_pool(name="lps", bufs=2, space="PSUM") as lps,
            tc.tile_pool(name="idxps", bufs=2 if OPTS["idxps2"] else 1,
                         space="PSUM") as idxps,
            tc.tile_pool(name="stps", bufs=1, space="PSUM") as stps,
            tc.tile_pool(name="yps", bufs=3 if OPTS.get("st_txps") else 2, space="PSUM") as yps,
            tc.tile_pool(name="txps", bufs=2, space="PSUM") as txps,
        ):
            # --- persistent tables (split loads so the DMA engine hands
            # pieces to the first tiles' compute as they arrive) ---
            ident = singles.tile([P, P], BF)
            make_identity(nc, ident[:])
            wsel = singles.tile([P, P], F32, tag="wsel")
            nc.gpsimd.dma_start(out=wsel[:], in_=wsel_d[:])
            m8 = singles.tile([P, 8], F32, tag="m8")
            nc.gpsimd.dma_start(out=m8[:], in_=m8_d[:])
            iota_f = singles.tile([P, SH_PAD], F32)
            nc.gpsimd.iota(iota_f[:], pattern=[[1, SH_PAD]], base=0,
                           channel_multiplier=0,
                           allow_small_or_imprecise_dtypes=True)
            w1t_sb = singles.tile([P, CH, SH_PAD], BF)
            w2sh_sb = singles.tile([P, SH_CH, OUT], F8E3)

            state = {}

            def load_w1t_chunk(r, n=4):
                cw = CH // n
                nc.sync.dma_start(
                    out=w1t_sb[:, r * cw:(r + 1) * cw, :],
                    in_=w1t_sh_r[:, r * cw:(r + 1) * cw, :])

            def load_w2sh_chunk(r):
                nc.sync.dma_start(
                    out=w2sh_sb[:, r * 2:(r + 1) * 2, :],
                    in_=w2_sh_r[:, r * 2:(r + 1) * 2, :])

            def s0_load(t, eng=None):
                eng = eng or nc.sync
                xtok = xtokp.tile([P, IN], BF, tag="xtok")
                nx = OPTS.get("x_pieces", 2)
                w = IN // nx
                for i in range(nx):
                    eng.dma_start(out=xtok[:, i * w:(i + 1) * w],
                                  in_=x_tm[ts(t, P), i * w:(i + 1) * w])
                state[t] = {"xtok": xtok}

            def s0_transpose(t):
                stt = state[t]
                xtok = stt["xtok"]
                xt = xpool.tile([P, CH, P], BF, tag="xt")
                for r in range(CH // TR):
                    tx_ps = txps.tile([P, TR, P], BF, tag="txps")
                    for j in range(TR):
                        nc.tensor.transpose(tx_ps[:, j, :],
                                            xtok[:, ts(r * TR + j, P)],
                                            ident[:])
                    nc.scalar.copy(out=xt[:, r * TR:(r + 1) * TR, :],
                                   in_=tx_ps[:])
                stt["xt"] = xt

            def s0_logits(t):
                stt = state[t]
                xt = stt.pop("xt")
                l_ps = lps.tile([P, SH_PAD], F32)
                for c in range(CH):
                    nc.tensor.matmul(l_ps[:], lhsT=xt[:, c, :],
                                     rhs=w1t_sb[:, c, :],
                                     start=(c == 0), stop=(c == CH - 1))
                l_sb = spool.tile([P, SH_PAD], F32, tag="lsb")
                nc.vector.tensor_copy(out=l_sb[:], in_=l_ps[:])
                stt["l_sb"] = l_sb

            def s1_shallow(t):
                # shallow routing over dense logits (DVE only; gelu + S^T
                # deferred to st_build at slot end)
                stt = state[t]
                l_sb = stt["l_sb"]
                ml = spool.tile([P, SH_PAD], BF, tag="ml")
                nc.vector.memset(ml[:, SH_NODES:SH_PAD], 0.0)
                node = small.tile([P, 1], F32, tag="node")
                nc.vector.memset(node[:], 0.0)
                for d in range(SH_LV):
                    lo, w = 2**d - 1, 2**d
                    logit = small.tile([P, 1], F32, tag="logit")
                    nc.vector.scalar_tensor_tensor(
                        out=ml[:, lo:lo + w],
                        in0=iota_f[:, lo:lo + w],
                        scalar=node[:, :1],
                        in1=l_sb[:, lo:lo + w],
                        op0=OP.is_equal, op1=OP.mult,
                        accum_out=logit[:, :1])
                    b1 = small.tile([P, 1], F32, tag="b1")
                    nc.vector.tensor_scalar(
                        out=b1[:], in0=logit[:], scalar1=0.0, scalar2=1.0,
                        op0=OP.is_gt, op1=OP.add)
                    nc.vector.scalar_tensor_tensor(
                        out=node[:], in0=node[:], scalar=2.0, in1=b1[:],
                        op0=OP.mult, op1=OP.add)
                stt["ml"] = ml
                stt["node"] = node
                stt["diag_t"] = {}
                stt["w2g_t"] = {}

            def st_build(t):
                # S = gelu(ML); S^T chunks for next slot's combine
                stt = state[t]
                ml = stt.pop("ml")
                gl = ml
                nc.scalar.activation(out=gl[:], in_=ml[:],
                                     func=AF.Gelu_apprx_tanh)
                if OPTS["idxps2"] or OPTS.get("st_txps"):
                    st_ps = txps.tile([P, TR, P], BF, tag="txps")
                else:
                    st_ps = stps.tile([P, SH_CH, P], BF, tag="st_ps")
                for j in range(SH_CH):
                    nc.tensor.transpose(st_ps[:, j, :], gl[:, ts(j, P)],
                                        ident[:])
                st_sb = spool.tile([P, SH_CH, P], BF, tag="stsb")
                nc.scalar.copy(out=st_sb[:], in_=st_ps[:, 0:SH_CH, :])
                stt["st_sb"] = st_sb

            def deep_issue(t, d):
                # idx from node via PE select trick, then one full-row w1
                # gather (bf16 for 9/10, fp8 for the last level) and the
                # level's w2 fp8 gather immediately behind it.
                stt = state[t]
                node = stt["node"]
                rhs8 = small.tile([P, 8], F32, tag="rhs8")
                nc.vector.tensor_scalar(out=rhs8[:], in0=m8[:],
                                        scalar1=node[:, :1],
                                        scalar2=None, op0=OP.mult)
                idx_ps = idxps.tile([P, 8], F32, tag="idxps")
                nc.tensor.matmul(idx_ps[:], lhsT=wsel[:], rhs=rhs8[:],
                                 start=True, stop=True)
                # idx copy: gpsimd variant lands directly before the gather
                # in the same Pool queue (no DVE round-trip)
                idx = small.tile([P, P // 16], I16, tag="idx")
                if OPTS["idxcopy_pool"]:
                    nc.gpsimd.tensor_copy(out=idx[:], in_=idx_ps[:])
                else:
                    nc.vector.tensor_copy(out=idx[:], in_=idx_ps[:])
                if d < LAST_LV or not LVL11_FP8:
                    wp = w1gp.tile([P, 1, IN], BF, tag="w1g")
                    nc.gpsimd.dma_gather(
                        wp[:], w1s[:, :], idx[:, :], P, P, IN,
                        transpose=False, queue_num=qn())
                else:
                    wp = w1g8p.tile([P, 1, IN], F8E3, tag="w1g8")
                    nc.gpsimd.dma_gather(
                        wp[:], w1q8[:, :], idx[:, :], P, P, IN,
                        transpose=False, queue_num=qn())
                stt["w1g"] = wp
                if d == SH_LV and OPTS["early9"]:
                    # w2 row gather for level 9 is deferred to next slot's
                    # start (not chain-critical; keeps its buffer lifetime
                    # short when level-9 w1 gathers launch a slot early)
                    stt["idx9"] = idx
                else:
                    issue_w2(t, d, idx)

            def issue_w2(t, d, idx):
                stt = state[t]
                w2g = w2gp.tile([P, 1, IN], F8E3, tag="w2g")
                nc.gpsimd.dma_gather(
                    w2g[:], w2q[:, :], idx[:], P, P, IN,
                    transpose=False, queue_num=qn())
                stt["w2g_t"][d] = w2g

            def deep_consume(t, d):
                stt = state[t]
                node, xtok = stt["node"], stt["xtok"]
                wp = stt.pop("w1g")
                col = d - SH_LV
                if col == 0:
                    lg = gsave.tile([P, 4], F32, tag="lg")
                    stt["lg"] = lg
                lg = stt["lg"]
                # levels 9/10 route on the logit sign, so they use the
                # fused scalar_tensor_tensor dot (slow 1x DVE mode but the
                # accumulator matches the reference's precision; summing
                # bf16-rounded products flips ~6 routing bits per level).
                # The last level is value-only, so it takes the fast path:
                # 2x-mode multiply + 4x-mode accumulate over bf16 products.
                if d < LAST_LV:
                    nc.vector.scalar_tensor_tensor(
                        out=wp[:, 0, :], in0=xtok[:, :],
                        scalar=1.0, in1=wp[:, 0, :], op0=OP.bypass,
                        op1=OP.mult, accum_out=lg[:, col:col + 1])
                else:
                    if LVL11_FP8:
                        # fp8 w1 row: products to a bf16 scratch (reuse
                        # the w1g ring); accumulates 128*<x,w1>.
                        prod = w1gp.tile([P, 1, IN], BF, tag="w1g")
                        prod_ap = prod[:, 0, :]
                        sc = 1.0 / W1SCALE
                    else:
                        prod_ap = wp[:, 0, :]
                        sc = 1.0
                    nc.vector.tensor_tensor(
                        out=prod_ap, in0=xtok[:, :], in1=wp[:, 0, :],
                        op=OP.mult)
                    nc.vector.tensor_scalar(
                        out=prod_ap, in0=prod_ap, scalar1=sc, scalar2=0.0,
                        op0=OP.mult, op1=OP.add,
                        accum_out=lg[:, col:col + 1])
                if d < LAST_LV:
                    b1 = small.tile([P, 1], F32, tag="b1")
                    nc.vector.tensor_scalar(
                        out=b1[:], in0=lg[:, col:col + 1], scalar1=0.0,
                        scalar2=1.0, op0=OP.is_gt, op1=OP.add)
                    nc.vector.scalar_tensor_tensor(
                        out=node[:], in0=node[:], scalar=2.0, in1=b1[:],
                        op0=OP.mult, op1=OP.add)

            def deep_finish(t):
                # one batched gelu over the 3 deep logits, then the three
                # fp8 diag builds for next slot's combine
                stt = state[t]
                lg = stt.pop("lg")
                g3 = gsave.tile([P, 4], F32, tag="g3")
                nc.scalar.activation(out=g3[:, 0:3], in_=lg[:, 0:3],
                                     func=AF.Gelu_apprx_tanh)
                for d in DEEP_LV:
                    dg = diagp.tile([P, P], F8E3, tag="diag")
                    nc.gpsimd.tensor_scalar(
                        out=dg[:], in0=ident[:],
                        scalar1=g3[:, d - SH_LV:d - SH_LV + 1],
                        scalar2=None, op0=OP.mult)
                    stt["diag_t"][d] = dg

            def s2(t, qlo=0, qhi=NQ):
                # y combine + store: per quarter, 3 deep diag fp8 matmuls +
                # 4 shallow S^T chunks against the fp8 shallow table.
                stt = state[t]
                st_sb = stt["st_sb"]
                diag_t, w2g_t = stt["diag_t"], stt["w2g_t"]
                for q in range(qlo, qhi):
                    y_ps = yps.tile([P, QW], F32)
                    col0 = q * QW
                    for j in range(SH_CH):
                        nc.tensor.matmul(
                            y_ps[:], lhsT=st_sb[:, j, :],
                            rhs=w2sh_sb[:, j, col0:col0 + QW],
                            start=(j == 0), stop=False)
                    for d in DEEP_LV:
                        nc.tensor.matmul(
                            y_ps[:], lhsT=diag_t[d][:],
                            rhs=w2g_t[d][:, 0, col0:col0 + QW],
                            start=False, stop=(d == LAST_LV))
                    y_sb = ypool.tile([P, QW], BF, tag="ysb")
                    nc.scalar.activation(out=y_sb[:], in_=y_ps[:],
                                         func=AF.Identity,
                                         scale=1.0 / W2SCALE)
                    yeng = nc.scalar if OPTS.get("y_on_act") else nc.sync
                    yeng.dma_start(out=y[ts(t, P), ts(q, QW)],
                                   in_=y_sb[:])

            def emit_slot(m):
                gm1 = groups[m - 1] if 1 <= m <= ng else []
                gm2 = groups[m - 2] if 2 <= m <= ng + 1 else []
                gm0 = groups[m] if m < ng else []
                gp1 = groups[m + 1] if m + 1 < ng else []

                # combine work of group m-2, emitted in chunks as PE cover:
                # each batch lands BEFORE the next ladder level's idx
                # matmuls, so the PE queue head is ready work while the
                # previous level's gather+dot latency resolves
                if OPTS["early9"]:
                    comb = [(t, q, q + 1) for t in gm2 for q in range(NQ)]
                else:
                    comb = [(t, ql, qh) for t in gm2
                            for (ql, qh) in OPTS["comb_chunks"]]

                def emit_comb(n):
                    for _ in range(n):
                        if comb:
                            s2(*comb.pop(0))

                if OPTS["early9"]:
                    # group m-1's level-9 gathers were issued at the END of
                    # the previous slot (right after its routing), so this
                    # slot's ladder is only the level-10/11 chain.
                    for t in gm1:
                        issue_w2(t, SH_LV, state[t].pop("idx9"))
                    for t in gm1:
                        st_build(t)
                    QUS = 1.5              # PE us per combine quarter
                    LAT = 11.0             # per-level ladder chain latency
                    STAG = 3.0             # per-tile dot stagger on DVE
                    sched = []
                    for lv_i, d in enumerate(DEEP_LV[1:], start=1):
                        for t_i, t in enumerate(gm1):
                            sched.append(
                                (4.0 + (lv_i - 1) * LAT + t_i * STAG,
                                 d, t_i, t))
                    sched.sort()
                    pos = 0.0
                    for when, d, t_i, t in sched:
                        want = int(max(0.0, when - pos) / QUS + 0.5)
                        nq_emit = min(want, len(comb))
                        emit_comb(nq_emit)
                        pos += nq_emit * QUS
                        deep_consume(t, d - 1)
                        deep_issue(t, d)
                    emit_comb(len(comb))
                    for t in gm1:
                        deep_consume(t, LAST_LV)
                    for t in gm1:
                        deep_finish(t)
                else:
                    # classic: ladder fully inside this slot, level-major,
                    # combine chunks of group m-2 between levels
                    emit_comb(1)
                    for t in gm1:
                        deep_issue(t, SH_LV)
                    nlv = len(DEEP_LV)
                    for lv_i, d in enumerate(DEEP_LV):
                        if d > SH_LV:
                            left = nlv - lv_i + 1
                            emit_comb(max(1, len(comb) // left))
                            for t in gm1:
                                deep_issue(t, d)
                        for t in gm1:
                            deep_consume(t, d)
                    emit_comb(len(comb))
                    for t in gm1:
                        deep_finish(t)
                    for t in gm1:
                        st_build(t)
                for t in gm2:
                    state.pop(t, None)
                # next group's transposes + logits + its shallow routing,
                # so the next slot's ladder starts without routing latency
                for t in gm0:
                    s0_transpose(t)
                    s0_logits(t)
                for t in gm0:
                    s1_shallow(t)
                if OPTS["early9"]:
                    for t in gm0:
                        deep_issue(t, SH_LV)
                # x prefetch for group m+1
                for t in gp1:
                    s0_load(t)

            # prologue: x for group 0 interleaved with the shallow tables
            nw1t = OPTS.get("w1t_pieces", 4)
            s0_load(groups[0][0], eng=nc.sync)
            load_w1t_chunk(0, nw1t)
            for t in groups[0][1:]:
                s0_load(t, eng=nc.sync)
            for r in range(1, nw1t):
                load_w1t_chunk(r, nw1t)
            for r in range(2):
                load_w2sh_chunk(r)
            for m in range(ng + 2):
                emit_slot(m)

    nc.compile()
    return nc


_CACHED = {}


def _get_program(n_tiles=NT, num_devices=N_CORES):
    key = (n_tiles, num_devices)
    if key not in _CACHED:
        _CACHED[key] = build_program(n_tiles, num_devices)
    return _CACHED[key]


def idx_masks():
    i = np.arange(P)
    wsel = (i[:, None] % 16 == i[None, :] % 16).astype(np.float32)
    m8 = (i[:, None] // 16 == np.arange(8)[None, :]).astype(np.float32)
    return wsel, m8


def prep_inputs(input, w1s, w2s):
    """Host-side layout prep shared by all cores."""
    w1 = np.asarray(w1s)
    w1t_sh = np.zeros((IN, SH_PAD), dtype=w1.dtype)
    w1t_sh[:, :SH_NODES] = w1[:SH_NODES].T
    w1q8 = (w1.astype(np.float32) * W1SCALE).astype(ml_dtypes.float8_e3m4)
    w2q = (np.asarray(w2s).astype(np.float32) * W2SCALE).astype(
        ml_dtypes.float8_e3m4)
    return np.ascontiguousarray(w1t_sh), w1q8, w2q


def _run(input, w1s, w2s, **spmd_kwargs):
    from concourse.bass_utils import run_bass_kernel_spmd

    nc = _get_program()
    w1t_sh, w1q8, w2q = prep_inputs(input, w1s, w2s)
    w1 = np.ascontiguousarray(np.asarray(w1s))
    wsel, m8 = idx_masks()
    in_maps = []
    for c in range(N_CORES):
        im = {
            "x": np.ascontiguousarray(np.asarray(input)[c * TOK:(c + 1) * TOK]),
            "w1t_sh": w1t_sh,
            "w1s": w1,
            "w1q8": w1q8,
            "w2q": w2q,
            "wsel": wsel,
            "m8": m8,
        }
        in_maps.append(im)
    res = run_bass_kernel_spmd(nc, in_maps, core_ids=list(range(N_CORES)),
                               **spmd_kwargs)
    out = np.concatenate([res.results[c]["y"] for c in range(N_CORES)],
                         axis=0)
    return out.astype(ml_dtypes.bfloat16), res


def kernel(input, w1s, w2s, depth):
    assert int(depth) == DEPTH
    out, _ = _run(input, w1s, w2s)
    return out


# revision 64
# speedup vs baseline: 1.0912x; 1.0044x over previous
"""Fast Feedforward (FFF) tree-routing kernel for Trainium2, 8 NeuronCores.

Problem: B=8192 tokens, d=4096, binary tree depth 12 (4095 nodes).
Per token, per level: logit = <x, w1s[node]>; y += gelu(logit) * w2s[node];
node = 2*node + 1 + (logit > 0).

Strategy (data-parallel over tokens, 1024 tokens/core, 8 tiles of 128):
- Levels 0-8 (511 nodes): dense logits L = x @ W1[0:511]^T via PE matmul
  (xT built on-chip with PE transposes). Routing = per-level select/compare
  on L (DVE). Masked gelu'd logits S^T combine against an fp8 e3m4 shallow
  w2 table (sliced from the full w2q table, pre-scaled by 128).
- Levels 9-11: per-tile full-row w1/w2 dma_gathers (w1 bf16, w2 fp8).
  Levels 9/10 dot via the fused scalar_tensor_tensor (slower 1x DVE mode
  but reference-matching fp32 accumulation - summing bf16-rounded
  products flips routing bits); level 11 is value-only so it uses the
  fast 2x-multiply + 4x-accumulate pair. One batched gelu per tile at
  ladder end; y folds in via diag(gelu) fp8 matmuls. PSUM carries 128*y;
  the final copy divides by 128 (exact exponent shift).
- 4-group software pipeline (2 tiles/group): slot m runs combine(m-2)
  split into chunks emitted between ladder levels of group m-1 (so the
  PE queue head is always ready work while gather+dot latency resolves),
  then transposes+logits+routing(m) and x prefetch(m+1). Table loads are
  split into ~1MB pieces interleaved with the first x tiles so the first
  logits matmuls chase the arriving w1t chunks.
"""

import numpy as np
import ml_dtypes

import concourse.bacc as bacc
import concourse.bass as bass
import concourse.mybir as mybir
import concourse.tile as tile
from concourse.bass import ts
from concourse.masks import make_identity

P = 128
IN = 4096
OUT = 4096
DEPTH = 12
N_NODES = 2**DEPTH - 1          # 4095
N_CORES = 8
B = 8192
TOK = B // N_CORES              # 1024 tokens per core
NT = TOK // P                   # 8 tiles of 128 tokens
CH = IN // P                    # 32 feature chunks
TR = 8                          # transpose chunks per PSUM round
SH_LV = 9                       # dense shallow levels 0..8
SH_NODES = 2**SH_LV - 1         # 511
SH_PAD = 512
SH_CH = SH_PAD // P             # 4 node chunks for shallow combine
DEEP_LV = list(range(SH_LV, DEPTH))   # [9, 10, 11]
LAST_LV = DEPTH - 1
NQ = 8                          # y feature quarters
QW = OUT // NQ                  # 512
BF = mybir.dt.bfloat16
F32 = mybir.dt.float32
I16 = mybir.dt.int16
F8E3 = mybir.dt.float8e3
AF = mybir.ActivationFunctionType
OP = mybir.AluOpType

W2SCALE = 128.0                 # fp8 w2 table pre-scale (exact pow2)
W1SCALE = 128.0                 # fp8 deep-w1 table pre-scale (exact pow2)
LVL11_FP8 = False               # gather last-level w1 rows as fp8 e3m4
CUTS = [0, 2, 4, 6, NT]         # pipeline groups

# emission-order tuning knobs (swept during development)
OPTS = {
    "early9": False,        # launch level-9 gathers at end of prior slot
    "idxcopy_pool": False,  # idx PSUM->SBUF copy on gpsimd vs DVE
    "idxps2": False,        # 2 idx PSUM banks (st_build then shares txps)
    "comb_chunks": ((0, 2), (2, 5), (5, 8)),
    "x_pieces": 4,          # x tile loaded in 4 slices (less DMA blocking)
    "half_gather": True,    # levels 9/10: two half-row gathers + half-dots
}


def build_program(n_tiles=NT, num_devices=N_CORES):
    nc = bacc.Bacc("TRN2", target_bir_lowering=False, debug=False,
                   num_devices=num_devices, num_swdge_queues=4)
    x_tm = nc.dram_tensor("x", [n_tiles * P, IN], BF, kind="ExternalInput")
    w1t_sh = nc.dram_tensor("w1t_sh", [IN, SH_PAD], BF, kind="ExternalInput")
    w1s = nc.dram_tensor("w1s", [N_NODES, IN], BF, kind="ExternalInput")
    w1q8 = nc.dram_tensor("w1q8", [N_NODES, IN], F8E3, kind="ExternalInput")
    w2q = nc.dram_tensor("w2q", [N_NODES, IN], F8E3, kind="ExternalInput")
    y = nc.dram_tensor("y", [n_tiles * P, OUT], BF, kind="ExternalOutput")
    wsel_d = nc.dram_tensor("wsel", [P, P], F32, kind="ExternalInput")
    m8_d = nc.dram_tensor("m8", [P, 8], F32, kind="ExternalInput")

    w1t_sh_r = w1t_sh.rearrange("(c p) n -> p c n", p=P)   # [128, 32, 512]
    w2_sh_r = w2q[0:SH_PAD, :].rearrange("(j p) f -> p j f", p=P)

    qn_counter = [0]

    def qn():
        q = qn_counter[0] % 4
        qn_counter[0] += 1
        return q

    groups = [list(range(CUTS[i], CUTS[i + 1])) for i in range(len(CUTS) - 1)]
    ng = len(groups)

    with tile.TileContext(nc) as tc:
        with (
            tc.tile_pool(name="singles", bufs=1) as singles,
            tc.tile_pool(name="xpool", bufs=2) as xpool,
            tc.tile_pool(name="xtokpool", bufs=OPTS.get("xtok_bufs", 5)) as xtokp,
            tc.tile_pool(name="spool", bufs=3) as spool,
            tc.tile_pool(name="small", bufs=16) as small,
            tc.tile_pool(name="w2gpool", bufs=9) as w2gp,
            tc.tile_pool(name="diagpool", bufs=12) as diagp,
            tc.tile_pool(name="w1gpool", bufs=5) as w1gp,
            tc.tile_pool(name="w1g8pool", bufs=2) as w1g8p,
            tc.tile_pool(name="gsave", bufs=12) as gsave,
            tc.tile_pool(name="ypool", bufs=8) as ypool,
            tc.tile_pool(name="lps", bufs=2, space="PSUM") as lps,
            tc.tile_pool(name="idxps", bufs=2 if OPTS["idxps2"] else 1,
                         space="PSUM") as idxps,
            tc.tile_pool(name="stps", bufs=1, space="PSUM") as stps,
            tc.tile_pool(name="yps", bufs=3 if OPTS.get("st_txps") else 2, space="PSUM") as yps,
            tc.tile_pool(name="txps", bufs=2, space="PSUM") as txps,
        ):
            # --- persistent tables (split loads so the DMA engine hands
            # pieces to the first tiles' compute as they arrive) ---
            ident = singles.tile([P, P], BF)
            make_identity(nc, ident[:])
            wsel = singles.tile([P, P], F32, tag="wsel")
            nc.gpsimd.dma_start(out=wsel[:], in_=wsel_d[:])
            m8 = singles.tile([P, 8], F32, tag="m8")
            nc.gpsimd.dma_start(out=m8[:], in_=m8_d[:])
            iota_f = singles.tile([P, SH_PAD], F32)
            nc.gpsimd.iota(iota_f[:], pattern=[[1, SH_PAD]], base=0,
                           channel_multiplier=0,
                           allow_small_or_imprecise_dtypes=True)
            w1t_sb = singles.tile([P, CH, SH_PAD], BF)
            w2sh_sb = singles.tile([P, SH_CH, OUT], F8E3)

            state = {}

            def load_w1t_chunk(r, n=4):
                cw = CH // n
                nc.sync.dma_start(
                    out=w1t_sb[:, r * cw:(r + 1) * cw, :],
                    in_=w1t_sh_r[:, r * cw:(r + 1) * cw, :])

            def load_w2sh_chunk(r):
                nc.sync.dma_start(
                    out=w2sh_sb[:, r * 2:(r + 1) * 2, :],
                    in_=w2_sh_r[:, r * 2:(r + 1) * 2, :])

            def s0_load(t, eng=None):
                eng = eng or nc.sync
                xtok = xtokp.tile([P, IN], BF, tag="xtok")
                nx = OPTS.get("x_pieces", 2)
                w = IN // nx
                for i in range(nx):
                    eng.dma_start(out=xtok[:, i * w:(i + 1) * w],
                                  in_=x_tm[ts(t, P), i * w:(i + 1) * w])
                state[t] = {"xtok": xtok}

            def s0_transpose(t):
                stt = state[t]
                xtok = stt["xtok"]
                xt = xpool.tile([P, CH, P], BF, tag="xt")
                for r in range(CH // TR):
                    tx_ps = txps.tile([P, TR, P], BF, tag="txps")
                    for j in range(TR):
                        nc.tensor.transpose(tx_ps[:, j, :],
                                            xtok[:, ts(r * TR + j, P)],
                                            ident[:])
                    nc.scalar.copy(out=xt[:, r * TR:(r + 1) * TR, :],
                                   in_=tx_ps[:])
                stt["xt"] = xt

            def s0_logits(t):
                stt = state[t]
                xt = stt.pop("xt")
                l_ps = lps.tile([P, SH_PAD], F32)
                for c in range(CH):
                    nc.tensor.matmul(l_ps[:], lhsT=xt[:, c, :],
                                     rhs=w1t_sb[:, c, :],
                                     start=(c == 0), stop=(c == CH - 1))
                l_sb = spool.tile([P, SH_PAD], F32, tag="lsb")
                nc.vector.tensor_copy(out=l_sb[:], in_=l_ps[:])
                stt["l_sb"] = l_sb

            def s1_shallow(t):
                # shallow routing over dense logits (DVE only; gelu + S^T
                # deferred to st_build at slot end)
                stt = state[t]
                l_sb = stt["l_sb"]
                ml = spool.tile([P, SH_PAD], BF, tag="ml")
                nc.vector.memset(ml[:, SH_NODES:SH_PAD], 0.0)
                node = small.tile([P, 1], F32, tag="node")
                nc.vector.memset(node[:], 0.0)
                for d in range(SH_LV):
                    lo, w = 2**d - 1, 2**d
                    logit = small.tile([P, 1], F32, tag="logit")
                    nc.vector.scalar_tensor_tensor(
                        out=ml[:, lo:lo + w],
                        in0=iota_f[:, lo:lo + w],
                        scalar=node[:, :1],
                        in1=l_sb[:, lo:lo + w],
                        op0=OP.is_equal, op1=OP.mult,
                        accum_out=logit[:, :1])
                    b1 = small.tile([P, 1], F32, tag="b1")
                    nc.vector.tensor_scalar(
                        out=b1[:], in0=logit[:], scalar1=0.0, scalar2=1.0,
                        op0=OP.is_gt, op1=OP.add)
                    nc.vector.scalar_tensor_tensor(
                        out=node[:], in0=node[:], scalar=2.0, in1=b1[:],
                        op0=OP.mult, op1=OP.add)
                stt["ml"] = ml
                stt["node"] = node
                stt["diag_t"] = {}
                stt["w2g_t"] = {}

            def st_build(t):
                # S = gelu(ML); S^T chunks for next slot's combine
                stt = state[t]
                ml = stt.pop("ml")
                gl = ml
                nc.scalar.activation(out=gl[:], in_=ml[:],
                                     func=AF.Gelu_apprx_tanh)
                if OPTS["idxps2"] or OPTS.get("st_txps"):
                    st_ps = txps.tile([P, TR, P], BF, tag="txps")
                else:
                    st_ps = stps.tile([P, SH_CH, P], BF, tag="st_ps")
                for j in range(SH_CH):
                    nc.tensor.transpose(st_ps[:, j, :], gl[:, ts(j, P)],
                                        ident[:])
                st_sb = spool.tile([P, SH_CH, P], BF, tag="stsb")
                nc.scalar.copy(out=st_sb[:], in_=st_ps[:, 0:SH_CH, :])
                stt["st_sb"] = st_sb

            def deep_issue(t, d):
                # idx from node via PE select trick, then one full-row w1
                # gather (bf16 for 9/10, fp8 for the last level) and the
                # level's w2 fp8 gather immediately behind it.
                stt = state[t]
                node = stt["node"]
                rhs8 = small.tile([P, 8], F32, tag="rhs8")
                nc.vector.tensor_scalar(out=rhs8[:], in0=m8[:],
                                        scalar1=node[:, :1],
                                        scalar2=None, op0=OP.mult)
                idx_ps = idxps.tile([P, 8], F32, tag="idxps")
                nc.tensor.matmul(idx_ps[:], lhsT=wsel[:], rhs=rhs8[:],
                                 start=True, stop=True)
                # idx copy: gpsimd variant lands directly before the gather
                # in the same Pool queue (no DVE round-trip)
                idx = small.tile([P, P // 16], I16, tag="idx")
                if OPTS["idxcopy_pool"]:
                    nc.gpsimd.tensor_copy(out=idx[:], in_=idx_ps[:])
                else:
                    nc.vector.tensor_copy(out=idx[:], in_=idx_ps[:])
                if d < LAST_LV and OPTS.get("half_gather"):
                    # routing levels: two half-row gathers so the first
                    # half-dot can overlap the second half's transfer
                    wp = w1gp.tile([P, 1, IN], BF, tag="w1g")
                    H = IN // 2
                    for h in range(2):
                        nc.gpsimd.dma_gather(
                            wp[:, :, h * H:(h + 1) * H],
                            w1s[:, h * H:(h + 1) * H], idx[:, :],
                            P, P, H, elem_step=IN,
                            transpose=False, queue_num=qn())
                elif d < LAST_LV or not LVL11_FP8:
                    wp = w1gp.tile([P, 1, IN], BF, tag="w1g")
                    nc.gpsimd.dma_gather(
                        wp[:], w1s[:, :], idx[:, :], P, P, IN,
                        transpose=False, queue_num=qn())
                else:
                    wp = w1g8p.tile([P, 1, IN], F8E3, tag="w1g8")
                    nc.gpsimd.dma_gather(
                        wp[:], w1q8[:, :], idx[:, :], P, P, IN,
                        transpose=False, queue_num=qn())
                stt["w1g"] = wp
                if d == SH_LV and OPTS["early9"]:
                    # w2 row gather for level 9 is deferred to next slot's
                    # start (not chain-critical; keeps its buffer lifetime
                    # short when level-9 w1 gathers launch a slot early)
                    stt["idx9"] = idx
                else:
                    issue_w2(t, d, idx)

            def issue_w2(t, d, idx):
                stt = state[t]
                w2g = w2gp.tile([P, 1, IN], F8E3, tag="w2g")
                nc.gpsimd.dma_gather(
                    w2g[:], w2q[:, :], idx[:], P, P, IN,
                    transpose=False, queue_num=qn())
                stt["w2g_t"][d] = w2g

            def deep_consume(t, d):
                stt = state[t]
                node, xtok = stt["node"], stt["xtok"]
                wp = stt.pop("w1g")
                col = d - SH_LV
                if col == 0:
                    lg = gsave.tile([P, 6], F32, tag="lg")
                    stt["lg"] = lg
                lg = stt["lg"]
                # levels 9/10 route on the logit sign, so they use the
                # fused scalar_tensor_tensor dot (slow 1x DVE mode but the
                # accumulator matches the reference's precision; summing
                # bf16-rounded products flips ~6 routing bits per level).
                # The last level is value-only, so it takes the fast path:
                # 2x-mode multiply + 4x-mode accumulate over bf16 products.
                if d < LAST_LV and OPTS.get("half_gather"):
                    H = IN // 2
                    for h in range(2):
                        nc.vector.scalar_tensor_tensor(
                            out=wp[:, 0, h * H:(h + 1) * H],
                            in0=xtok[:, h * H:(h + 1) * H],
                            scalar=1.0, in1=wp[:, 0, h * H:(h + 1) * H],
                            op0=OP.bypass, op1=OP.mult,
                            accum_out=lg[:, 4 + h:5 + h])
                    nc.vector.tensor_tensor(
                        out=lg[:, col:col + 1], in0=lg[:, 4:5],
                        in1=lg[:, 5:6], op=OP.add)
                elif d < LAST_LV:
                    nc.vector.scalar_tensor_tensor(
                        out=wp[:, 0, :], in0=xtok[:, :],
                        scalar=1.0, in1=wp[:, 0, :], op0=OP.bypass,
                        op1=OP.mult, accum_out=lg[:, col:col + 1])
                else:
                    if LVL11_FP8:
                        # fp8 w1 row: products to a bf16 scratch (reuse
                        # the w1g ring); accumulates 128*<x,w1>.
                        prod = w1gp.tile([P, 1, IN], BF, tag="w1g")
                        prod_ap = prod[:, 0, :]
                        sc = 1.0 / W1SCALE
                    else:
                        prod_ap = wp[:, 0, :]
                        sc = 1.0
                    nc.vector.tensor_tensor(
                        out=prod_ap, in0=xtok[:, :], in1=wp[:, 0, :],
                        op=OP.mult)
                    nc.vector.tensor_scalar(
                        out=prod_ap, in0=prod_ap, scalar1=sc, scalar2=0.0,
                        op0=OP.mult, op1=OP.add,
                        accum_out=lg[:, col:col + 1])
                if d < LAST_LV:
                    b1 = small.tile([P, 1], F32, tag="b1")
                    nc.vector.tensor_scalar(
                        out=b1[:], in0=lg[:, col:col + 1], scalar1=0.0,
                        scalar2=1.0, op0=OP.is_gt, op1=OP.add)
                    nc.vector.scalar_tensor_tensor(
                        out=node[:], in0=node[:], scalar=2.0, in1=b1[:],
                        op0=OP.mult, op1=OP.add)

            def deep_finish(t):
                # one batched gelu over the 3 deep logits, then the three
                # fp8 diag builds for next slot's combine
                stt = state[t]
                lg = stt.pop("lg")
                g3 = gsave.tile([P, 4], F32, tag="g3")
                nc.scalar.activation(out=g3[:, 0:3], in_=lg[:, 0:3],
                                     func=AF.Gelu_apprx_tanh)
                for d in DEEP_LV:
                    dg = diagp.tile([P, P], F8E3, tag="diag")
                    nc.gpsimd.tensor_scalar(
                        out=dg[:], in0=ident[:],
                        scalar1=g3[:, d - SH_LV:d - SH_LV + 1],
                        scalar2=None, op0=OP.mult)
                    stt["diag_t"][d] = dg

            def s2(t, qlo=0, qhi=NQ):
                # y combine + store: per quarter, 3 deep diag fp8 matmuls +
                # 4 shallow S^T chunks against the fp8 shallow table.
                stt = state[t]
                st_sb = stt["st_sb"]
                diag_t, w2g_t = stt["diag_t"], stt["w2g_t"]
                for q in range(qlo, qhi):
                    y_ps = yps.tile([P, QW], F32)
                    col0 = q * QW
                    for j in range(SH_CH):
                        nc.tensor.matmul(
                            y_ps[:], lhsT=st_sb[:, j, :],
                            rhs=w2sh_sb[:, j, col0:col0 + QW],
                            start=(j == 0), stop=False)
                    for d in DEEP_LV:
                        nc.tensor.matmul(
                            y_ps[:], lhsT=diag_t[d][:],
                            rhs=w2g_t[d][:, 0, col0:col0 + QW],
                            start=False, stop=(d == LAST_LV))
                    y_sb = ypool.tile([P, QW], BF, tag="ysb")
                    nc.scalar.activation(out=y_sb[:], in_=y_ps[:],
                                         func=AF.Identity,
                                         scale=1.0 / W2SCALE)
                    yeng = nc.scalar if OPTS.get("y_on_act") else nc.sync
                    yeng.dma_start(out=y[ts(t, P), ts(q, QW)],
                                   in_=y_sb[:])

            def emit_slot(m):
                gm1 = groups[m - 1] if 1 <= m <= ng else []
                gm2 = groups[m - 2] if 2 <= m <= ng + 1 else []
                gm0 = groups[m] if m < ng else []
                gp1 = groups[m + 1] if m + 1 < ng else []

                # combine work of group m-2, emitted in chunks as PE cover:
                # each batch lands BEFORE the next ladder level's idx
                # matmuls, so the PE queue head is ready work while the
                # previous level's gather+dot latency resolves
                if OPTS["early9"]:
                    comb = [(t, q, q + 1) for t in gm2 for q in range(NQ)]
                else:
                    comb = [(t, ql, qh) for t in gm2
                            for (ql, qh) in OPTS["comb_chunks"]]

                def emit_comb(n):
                    for _ in range(n):
                        if comb:
                            s2(*comb.pop(0))

                if OPTS["early9"]:
                    # group m-1's level-9 gathers were issued at the END of
                    # the previous slot (right after its routing), so this
                    # slot's ladder is only the level-10/11 chain.
                    for t in gm1:
                        issue_w2(t, SH_LV, state[t].pop("idx9"))
                    for t in gm1:
                        st_build(t)
                    QUS = 1.5              # PE us per combine quarter
                    LAT = 11.0             # per-level ladder chain latency
                    STAG = 3.0             # per-tile dot stagger on DVE
                    sched = []
                    for lv_i, d in enumerate(DEEP_LV[1:], start=1):
                        for t_i, t in enumerate(gm1):
                            sched.append(
                                (4.0 + (lv_i - 1) * LAT + t_i * STAG,
                                 d, t_i, t))
                    sched.sort()
                    pos = 0.0
                    for when, d, t_i, t in sched:
                        want = int(max(0.0, when - pos) / QUS + 0.5)
                        nq_emit = min(want, len(comb))
                        emit_comb(nq_emit)
                        pos += nq_emit * QUS
                        deep_consume(t, d - 1)
                        deep_issue(t, d)
                    emit_comb(len(comb))
                    for t in gm1:
                        deep_consume(t, LAST_LV)
                    for t in gm1:
                        deep_finish(t)
                else:
                    # classic: ladder fully inside this slot, level-major,
                    # combine chunks of group m-2 between levels
                    emit_comb(1)
                    for t in gm1:
                        deep_issue(t, SH_LV)
                    nlv = len(DEEP_LV)
                    for lv_i, d in enumerate(DEEP_LV):
                        if d > SH_LV:
                            left = nlv - lv_i + 1
                            emit_comb(max(1, len(comb) // left))
                            for t in gm1:
                                deep_issue(t, d)
                        for t in gm1:
                            deep_consume(t, d)
                    emit_comb(len(comb))
                    for t in gm1:
                        deep_finish(t)
                    for t in gm1:
                        st_build(t)
                for t in gm2:
                    state.pop(t, None)
                # next group's transposes + logits + its shallow routing,
                # so the next slot's ladder starts without routing latency
                for t in gm0:
                    s0_transpose(t)
                    s0_logits(t)
                for t in gm0:
                    s1_shallow(t)
                if OPTS["early9"]:
                    for t in gm0:
                        deep_issue(t, SH_LV)
                # x prefetch for group m+1
                for t in gp1:
                    s0_load(t)

            # prologue: x for group 0 interleaved with the shallow tables
            nw1t = OPTS.get("w1t_pieces", 4)
            s0_load(groups[0][0], eng=nc.sync)
            load_w1t_chunk(0, nw1t)
            for t in groups[0][1:]:
                s0_load(t, eng=nc.sync)
            for r in range(1, nw1t):
                load_w1t_chunk(r, nw1t)
            for r in range(2):
                load_w2sh_chunk(r)
            for m in range(ng + 2):
                emit_slot(m)

    nc.compile()
    return nc


_CACHED = {}


def _get_program(n_tiles=NT, num_devices=N_CORES):
    key = (n_tiles, num_devices)
    if key not in _CACHED:
        _CACHED[key] = build_program(n_tiles, num_devices)
    return _CACHED[key]


def idx_masks():
    i = np.arange(P)
    wsel = (i[:, None] % 16 == i[None, :] % 16).astype(np.float32)
    m8 = (i[:, None] // 16 == np.arange(8)[None, :]).astype(np.float32)
    return wsel, m8


def prep_inputs(input, w1s, w2s):
    """Host-side layout prep shared by all cores."""
    w1 = np.asarray(w1s)
    w1t_sh = np.zeros((IN, SH_PAD), dtype=w1.dtype)
    w1t_sh[:, :SH_NODES] = w1[:SH_NODES].T
    w1q8 = (w1.astype(np.float32) * W1SCALE).astype(ml_dtypes.float8_e3m4)
    w2q = (np.asarray(w2s).astype(np.float32) * W2SCALE).astype(
        ml_dtypes.float8_e3m4)
    return np.ascontiguousarray(w1t_sh), w1q8, w2q


def _run(input, w1s, w2s, **spmd_kwargs):
    from concourse.bass_utils import run_bass_kernel_spmd

    nc = _get_program()
    w1t_sh, w1q8, w2q = prep_inputs(input, w1s, w2s)
    w1 = np.ascontiguousarray(np.asarray(w1s))
    wsel, m8 = idx_masks()
    in_maps = []
    for c in range(N_CORES):
        im = {
            "x": np.ascontiguousarray(np.asarray(input)[c * TOK:(c + 1) * TOK]),
            "w1t_sh": w1t_sh,
            "w1s": w1,
            "w1q8": w1q8,
            "w2q": w2q,
            "wsel": wsel,
            "m8": m8,
        }
        in_maps.append(im)
    res = run_bass_kernel_spmd(nc, in_maps, core_ids=list(range(N_CORES)),
                               **spmd_kwargs)
    out = np.concatenate([res.results[c]["y"] for c in range(N_CORES)],
                         axis=0)
    return out.astype(ml_dtypes.bfloat16), res


def kernel(input, w1s, w2s, depth):
    assert int(depth) == DEPTH
    out, _ = _run(input, w1s, w2s)
    return out


# revision 68
# speedup vs baseline: 1.0973x; 1.0056x over previous
"""Fast Feedforward (FFF) tree-routing kernel for Trainium2, 8 NeuronCores.

Problem: B=8192 tokens, d=4096, binary tree depth 12 (4095 nodes).
Per token, per level: logit = <x, w1s[node]>; y += gelu(logit) * w2s[node];
node = 2*node + 1 + (logit > 0).

Strategy (data-parallel over tokens, 1024 tokens/core, 8 tiles of 128):
- Levels 0-8 (511 nodes): dense logits L = x @ W1[0:511]^T via PE matmul
  (xT built on-chip with PE transposes). Routing = per-level select/compare
  on L (DVE). Masked gelu'd logits S^T combine against an fp8 e3m4 shallow
  w2 table (sliced from the full w2q table, pre-scaled by 128).
- Levels 9-11: per-tile full-row w1/w2 dma_gathers (w1 bf16, w2 fp8).
  Levels 9/10 dot via the fused scalar_tensor_tensor (slower 1x DVE mode
  but reference-matching fp32 accumulation - summing bf16-rounded
  products flips routing bits); level 11 is value-only so it uses the
  fast 2x-multiply + 4x-accumulate pair. One batched gelu per tile at
  ladder end; y folds in via diag(gelu) fp8 matmuls. PSUM carries 128*y;
  the final copy divides by 128 (exact exponent shift).
- 4-group software pipeline (2 tiles/group): slot m runs combine(m-2)
  split into chunks emitted between ladder levels of group m-1 (so the
  PE queue head is always ready work while gather+dot latency resolves),
  then transposes+logits+routing(m) and x prefetch(m+1). Table loads are
  split into ~1MB pieces interleaved with the first x tiles so the first
  logits matmuls chase the arriving w1t chunks.
"""

import numpy as np
import ml_dtypes

import concourse.bacc as bacc
import concourse.bass as bass
import concourse.mybir as mybir
import concourse.tile as tile
from concourse.bass import ts
from concourse.masks import make_identity

P = 128
IN = 4096
OUT = 4096
DEPTH = 12
N_NODES = 2**DEPTH - 1          # 4095
N_CORES = 8
B = 8192
TOK = B // N_CORES              # 1024 tokens per core
NT = TOK // P                   # 8 tiles of 128 tokens
CH = IN // P                    # 32 feature chunks
TR = 8                          # transpose chunks per PSUM round
SH_LV = 9                       # dense shallow levels 0..8
SH_NODES = 2**SH_LV - 1         # 511
SH_PAD = 512
SH_CH = SH_PAD // P             # 4 node chunks for shallow combine
DEEP_LV = list(range(SH_LV, DEPTH))   # [9, 10, 11]
LAST_LV = DEPTH - 1
NQ = 8                          # y feature quarters
QW = OUT // NQ                  # 512
BF = mybir.dt.bfloat16
F32 = mybir.dt.float32
I16 = mybir.dt.int16
F8E3 = mybir.dt.float8e3
AF = mybir.ActivationFunctionType
OP = mybir.AluOpType

W2SCALE = 128.0                 # fp8 w2 table pre-scale (exact pow2)
W1SCALE = 128.0                 # fp8 deep-w1 table pre-scale (exact pow2)
LVL11_FP8 = False               # gather last-level w1 rows as fp8 e3m4
CUTS = [0, 2, 4, 6, NT]         # pipeline groups

# emission-order tuning knobs (swept during development)
OPTS = {
    "early9": False,        # launch level-9 gathers at end of prior slot
    "idxcopy_pool": False,  # idx PSUM->SBUF copy on gpsimd vs DVE
    "idxps2": False,        # 2 idx PSUM banks (st_build then shares txps)
    "comb_chunks": ((0, 2), (2, 5), (5, 8)),
    "x_pieces": 4,          # x tile loaded in 4 slices (less DMA blocking)
    "half_gather": True,    # levels 9/10: two half-row gathers + half-dots
    "yps3": True,           # 3 combine PSUM banks (logits PSUM drops to 1)
}


def build_program(n_tiles=NT, num_devices=N_CORES):
    nc = bacc.Bacc("TRN2", target_bir_lowering=False, debug=False,
                   num_devices=num_devices,
                   num_swdge_queues=OPTS.get("nq", 4))
    x_tm = nc.dram_tensor("x", [n_tiles * P, IN], BF, kind="ExternalInput")
    w1t_sh = nc.dram_tensor("w1t_sh", [IN, SH_PAD], BF, kind="ExternalInput")
    w1s = nc.dram_tensor("w1s", [N_NODES, IN], BF, kind="ExternalInput")
    w1q8 = nc.dram_tensor("w1q8", [N_NODES, IN], F8E3, kind="ExternalInput")
    w2q = nc.dram_tensor("w2q", [N_NODES, IN], F8E3, kind="ExternalInput")
    y = nc.dram_tensor("y", [n_tiles * P, OUT], BF, kind="ExternalOutput")
    wsel_d = nc.dram_tensor("wsel", [P, P], F32, kind="ExternalInput")
    m8_d = nc.dram_tensor("m8", [P, 8], F32, kind="ExternalInput")

    w1t_sh_r = w1t_sh.rearrange("(c p) n -> p c n", p=P)   # [128, 32, 512]
    w2_sh_r = w2q[0:SH_PAD, :].rearrange("(j p) f -> p j f", p=P)

    qn_counter = [0]

    def qn():
        q = qn_counter[0] % OPTS.get("nq", 4)
        qn_counter[0] += 1
        return q

    groups = [list(range(CUTS[i], CUTS[i + 1])) for i in range(len(CUTS) - 1)]
    ng = len(groups)

    with tile.TileContext(nc) as tc:
        with (
            tc.tile_pool(name="singles", bufs=1) as singles,
            tc.tile_pool(name="xpool", bufs=2) as xpool,
            tc.tile_pool(name="xtokpool", bufs=OPTS.get("xtok_bufs", 5)) as xtokp,
            tc.tile_pool(name="spool", bufs=3) as spool,
            tc.tile_pool(name="small", bufs=16) as small,
            tc.tile_pool(name="w2gpool", bufs=9) as w2gp,
            tc.tile_pool(name="diagpool", bufs=12) as diagp,
            tc.tile_pool(name="w1gpool", bufs=5) as w1gp,
            tc.tile_pool(name="w1g8pool", bufs=2) as w1g8p,
            tc.tile_pool(name="gsave", bufs=12) as gsave,
            tc.tile_pool(name="ypool", bufs=8) as ypool,
            tc.tile_pool(name="lps", bufs=1 if OPTS.get("yps3") else 2, space="PSUM") as lps,
            tc.tile_pool(name="idxps", bufs=2 if OPTS["idxps2"] else 1,
                         space="PSUM") as idxps,
            tc.tile_pool(name="stps", bufs=1, space="PSUM") as stps,
            tc.tile_pool(name="yps", bufs=3 if (OPTS.get("st_txps") or OPTS.get("yps3")) else 2, space="PSUM") as yps,
            tc.tile_pool(name="txps", bufs=2, space="PSUM") as txps,
        ):
            # --- persistent tables (split loads so the DMA engine hands
            # pieces to the first tiles' compute as they arrive) ---
            ident = singles.tile([P, P], BF)
            make_identity(nc, ident[:])
            wsel = singles.tile([P, P], F32, tag="wsel")
            nc.gpsimd.dma_start(out=wsel[:], in_=wsel_d[:])
            m8 = singles.tile([P, 8], F32, tag="m8")
            nc.gpsimd.dma_start(out=m8[:], in_=m8_d[:])
            iota_f = singles.tile([P, SH_PAD], F32)
            nc.gpsimd.iota(iota_f[:], pattern=[[1, SH_PAD]], base=0,
                           channel_multiplier=0,
                           allow_small_or_imprecise_dtypes=True)
            w1t_sb = singles.tile([P, CH, SH_PAD], BF)
            w2sh_sb = singles.tile([P, SH_CH, OUT], F8E3)

            state = {}

            def load_w1t_chunk(r, n=4):
                cw = CH // n
                nc.sync.dma_start(
                    out=w1t_sb[:, r * cw:(r + 1) * cw, :],
                    in_=w1t_sh_r[:, r * cw:(r + 1) * cw, :])

            def load_w2sh_chunk(r):
                nc.sync.dma_start(
                    out=w2sh_sb[:, r * 2:(r + 1) * 2, :],
                    in_=w2_sh_r[:, r * 2:(r + 1) * 2, :])

            def s0_load(t, eng=None):
                eng = eng or nc.sync
                xtok = xtokp.tile([P, IN], BF, tag="xtok")
                nx = OPTS.get("x_pieces", 2)
                w = IN // nx
                for i in range(nx):
                    eng.dma_start(out=xtok[:, i * w:(i + 1) * w],
                                  in_=x_tm[ts(t, P), i * w:(i + 1) * w])
                state[t] = {"xtok": xtok}

            def s0_transpose(t):
                stt = state[t]
                xtok = stt["xtok"]
                xt = xpool.tile([P, CH, P], BF, tag="xt")
                for r in range(CH // TR):
                    tx_ps = txps.tile([P, TR, P], BF, tag="txps")
                    for j in range(TR):
                        nc.tensor.transpose(tx_ps[:, j, :],
                                            xtok[:, ts(r * TR + j, P)],
                                            ident[:])
                    nc.scalar.copy(out=xt[:, r * TR:(r + 1) * TR, :],
                                   in_=tx_ps[:])
                stt["xt"] = xt

            def s0_logits(t):
                stt = state[t]
                xt = stt.pop("xt")
                l_ps = lps.tile([P, SH_PAD], F32)
                for c in range(CH):
                    nc.tensor.matmul(l_ps[:], lhsT=xt[:, c, :],
                                     rhs=w1t_sb[:, c, :],
                                     start=(c == 0), stop=(c == CH - 1))
                l_sb = spool.tile([P, SH_PAD], F32, tag="lsb")
                nc.vector.tensor_copy(out=l_sb[:], in_=l_ps[:])
                stt["l_sb"] = l_sb

            def s1_shallow(t):
                # shallow routing over dense logits (DVE only; gelu + S^T
                # deferred to st_build at slot end)
                stt = state[t]
                l_sb = stt["l_sb"]
                ml = spool.tile([P, SH_PAD], BF, tag="ml")
                nc.vector.memset(ml[:, SH_NODES:SH_PAD], 0.0)
                node = small.tile([P, 1], F32, tag="node")
                nc.vector.memset(node[:], 0.0)
                for d in range(SH_LV):
                    lo, w = 2**d - 1, 2**d
                    logit = small.tile([P, 1], F32, tag="logit")
                    nc.vector.scalar_tensor_tensor(
                        out=ml[:, lo:lo + w],
                        in0=iota_f[:, lo:lo + w],
                        scalar=node[:, :1],
                        in1=l_sb[:, lo:lo + w],
                        op0=OP.is_equal, op1=OP.mult,
                        accum_out=logit[:, :1])
                    b1 = small.tile([P, 1], F32, tag="b1")
                    nc.vector.tensor_scalar(
                        out=b1[:], in0=logit[:], scalar1=0.0, scalar2=1.0,
                        op0=OP.is_gt, op1=OP.add)
                    nc.vector.scalar_tensor_tensor(
                        out=node[:], in0=node[:], scalar=2.0, in1=b1[:],
                        op0=OP.mult, op1=OP.add)
                stt["ml"] = ml
                stt["node"] = node
                stt["diag_t"] = {}
                stt["w2g_t"] = {}

            def st_build(t):
                # S = gelu(ML); S^T chunks for next slot's combine
                stt = state[t]
                ml = stt.pop("ml")
                gl = ml
                nc.scalar.activation(out=gl[:], in_=ml[:],
                                     func=AF.Gelu_apprx_tanh)
                if OPTS["idxps2"] or OPTS.get("st_txps"):
                    st_ps = txps.tile([P, TR, P], BF, tag="txps")
                else:
                    st_ps = stps.tile([P, SH_CH, P], BF, tag="st_ps")
                for j in range(SH_CH):
                    nc.tensor.transpose(st_ps[:, j, :], gl[:, ts(j, P)],
                                        ident[:])
                st_sb = spool.tile([P, SH_CH, P], BF, tag="stsb")
                nc.scalar.copy(out=st_sb[:], in_=st_ps[:, 0:SH_CH, :])
                stt["st_sb"] = st_sb

            def deep_issue(t, d):
                # idx from node via PE select trick, then one full-row w1
                # gather (bf16 for 9/10, fp8 for the last level) and the
                # level's w2 fp8 gather immediately behind it.
                stt = state[t]
                node = stt["node"]
                rhs8 = small.tile([P, 8], F32, tag="rhs8")
                nc.vector.tensor_scalar(out=rhs8[:], in0=m8[:],
                                        scalar1=node[:, :1],
                                        scalar2=None, op0=OP.mult)
                idx_ps = idxps.tile([P, 8], F32, tag="idxps")
                nc.tensor.matmul(idx_ps[:], lhsT=wsel[:], rhs=rhs8[:],
                                 start=True, stop=True)
                # idx copy: gpsimd variant lands directly before the gather
                # in the same Pool queue (no DVE round-trip)
                idx = small.tile([P, P // 16], I16, tag="idx")
                if OPTS["idxcopy_pool"]:
                    nc.gpsimd.tensor_copy(out=idx[:], in_=idx_ps[:])
                else:
                    nc.vector.tensor_copy(out=idx[:], in_=idx_ps[:])
                if d < LAST_LV and OPTS.get("half_gather"):
                    # routing levels: two half-row gathers so the first
                    # half-dot can overlap the second half's transfer
                    wp = w1gp.tile([P, 1, IN], BF, tag="w1g")
                    H = IN // 2
                    for h in range(2):
                        nc.gpsimd.dma_gather(
                            wp[:, :, h * H:(h + 1) * H],
                            w1s[:, h * H:(h + 1) * H], idx[:, :],
                            P, P, H, elem_step=IN,
                            transpose=False, queue_num=qn())
                elif d < LAST_LV or not LVL11_FP8:
                    wp = w1gp.tile([P, 1, IN], BF, tag="w1g")
                    nc.gpsimd.dma_gather(
                        wp[:], w1s[:, :], idx[:, :], P, P, IN,
                        transpose=False, queue_num=qn())
                else:
                    wp = w1g8p.tile([P, 1, IN], F8E3, tag="w1g8")
                    nc.gpsimd.dma_gather(
                        wp[:], w1q8[:, :], idx[:, :], P, P, IN,
                        transpose=False, queue_num=qn())
                stt["w1g"] = wp
                if d == SH_LV and OPTS["early9"]:
                    # w2 row gather for level 9 is deferred to next slot's
                    # start (not chain-critical; keeps its buffer lifetime
                    # short when level-9 w1 gathers launch a slot early)
                    stt["idx9"] = idx
                else:
                    issue_w2(t, d, idx)

            def issue_w2(t, d, idx):
                stt = state[t]
                w2g = w2gp.tile([P, 1, IN], F8E3, tag="w2g")
                nc.gpsimd.dma_gather(
                    w2g[:], w2q[:, :], idx[:], P, P, IN,
                    transpose=False, queue_num=qn())
                stt["w2g_t"][d] = w2g

            def deep_consume(t, d):
                stt = state[t]
                node, xtok = stt["node"], stt["xtok"]
                wp = stt.pop("w1g")
                col = d - SH_LV
                if col == 0:
                    lg = gsave.tile([P, 6], F32, tag="lg")
                    stt["lg"] = lg
                lg = stt["lg"]
                # levels 9/10 route on the logit sign, so they use the
                # fused scalar_tensor_tensor dot (slow 1x DVE mode but the
                # accumulator matches the reference's precision; summing
                # bf16-rounded products flips ~6 routing bits per level).
                # The last level is value-only, so it takes the fast path:
                # 2x-mode multiply + 4x-mode accumulate over bf16 products.
                if d < LAST_LV and OPTS.get("half_gather"):
                    H = IN // 2
                    for h in range(2):
                        nc.vector.scalar_tensor_tensor(
                            out=wp[:, 0, h * H:(h + 1) * H],
                            in0=xtok[:, h * H:(h + 1) * H],
                            scalar=1.0, in1=wp[:, 0, h * H:(h + 1) * H],
                            op0=OP.bypass, op1=OP.mult,
                            accum_out=lg[:, 4 + h:5 + h])
                    nc.vector.tensor_tensor(
                        out=lg[:, col:col + 1], in0=lg[:, 4:5],
                        in1=lg[:, 5:6], op=OP.add)
                elif d < LAST_LV:
                    nc.vector.scalar_tensor_tensor(
                        out=wp[:, 0, :], in0=xtok[:, :],
                        scalar=1.0, in1=wp[:, 0, :], op0=OP.bypass,
                        op1=OP.mult, accum_out=lg[:, col:col + 1])
                else:
                    if LVL11_FP8:
                        # fp8 w1 row: products to a bf16 scratch (reuse
                        # the w1g ring); accumulates 128*<x,w1>.
                        prod = w1gp.tile([P, 1, IN], BF, tag="w1g")
                        prod_ap = prod[:, 0, :]
                        sc = 1.0 / W1SCALE
                    else:
                        prod_ap = wp[:, 0, :]
                        sc = 1.0
                    nc.vector.tensor_tensor(
                        out=prod_ap, in0=xtok[:, :], in1=wp[:, 0, :],
                        op=OP.mult)
                    nc.vector.tensor_scalar(
                        out=prod_ap, in0=prod_ap, scalar1=sc, scalar2=0.0,
                        op0=OP.mult, op1=OP.add,
                        accum_out=lg[:, col:col + 1])
                if d < LAST_LV:
                    b1 = small.tile([P, 1], F32, tag="b1")
                    nc.vector.tensor_scalar(
                        out=b1[:], in0=lg[:, col:col + 1], scalar1=0.0,
                        scalar2=1.0, op0=OP.is_gt, op1=OP.add)
                    nc.vector.scalar_tensor_tensor(
                        out=node[:], in0=node[:], scalar=2.0, in1=b1[:],
                        op0=OP.mult, op1=OP.add)

            def deep_finish(t):
                # one batched gelu over the 3 deep logits, then the three
                # fp8 diag builds for next slot's combine
                stt = state[t]
                lg = stt.pop("lg")
                g3 = gsave.tile([P, 4], F32, tag="g3")
                nc.scalar.activation(out=g3[:, 0:3], in_=lg[:, 0:3],
                                     func=AF.Gelu_apprx_tanh)
                for d in DEEP_LV:
                    dg = diagp.tile([P, P], F8E3, tag="diag")
                    nc.gpsimd.tensor_scalar(
                        out=dg[:], in0=ident[:],
                        scalar1=g3[:, d - SH_LV:d - SH_LV + 1],
                        scalar2=None, op0=OP.mult)
                    stt["diag_t"][d] = dg

            def s2(t, qlo=0, qhi=NQ):
                # y combine + store: per quarter, 3 deep diag fp8 matmuls +
                # 4 shallow S^T chunks against the fp8 shallow table.
                stt = state[t]
                st_sb = stt["st_sb"]
                diag_t, w2g_t = stt["diag_t"], stt["w2g_t"]
                for q in range(qlo, qhi):
                    y_ps = yps.tile([P, QW], F32)
                    col0 = q * QW
                    for j in range(SH_CH):
                        nc.tensor.matmul(
                            y_ps[:], lhsT=st_sb[:, j, :],
                            rhs=w2sh_sb[:, j, col0:col0 + QW],
                            start=(j == 0), stop=False)
                    for d in DEEP_LV:
                        nc.tensor.matmul(
                            y_ps[:], lhsT=diag_t[d][:],
                            rhs=w2g_t[d][:, 0, col0:col0 + QW],
                            start=False, stop=(d == LAST_LV))
                    y_sb = ypool.tile([P, QW], BF, tag="ysb")
                    nc.scalar.activation(out=y_sb[:], in_=y_ps[:],
                                         func=AF.Identity,
                                         scale=1.0 / W2SCALE)
                    yeng = nc.scalar if OPTS.get("y_on_act") else nc.sync
                    yeng.dma_start(out=y[ts(t, P), ts(q, QW)],
                                   in_=y_sb[:])

            def emit_slot(m):
                gm1 = groups[m - 1] if 1 <= m <= ng else []
                gm2 = groups[m - 2] if 2 <= m <= ng + 1 else []
                gm0 = groups[m] if m < ng else []
                gp1 = groups[m + 1] if m + 1 < ng else []

                # combine work of group m-2, emitted in chunks as PE cover:
                # each batch lands BEFORE the next ladder level's idx
                # matmuls, so the PE queue head is ready work while the
                # previous level's gather+dot latency resolves
                if OPTS["early9"]:
                    comb = [(t, q, q + 1) for t in gm2 for q in range(NQ)]
                else:
                    comb = [(t, ql, qh) for t in gm2
                            for (ql, qh) in OPTS["comb_chunks"]]

                def emit_comb(n):
                    for _ in range(n):
                        if comb:
                            s2(*comb.pop(0))

                if OPTS["early9"]:
                    # group m-1's level-9 gathers were issued at the END of
                    # the previous slot (right after its routing), so this
                    # slot's ladder is only the level-10/11 chain.
                    for t in gm1:
                        issue_w2(t, SH_LV, state[t].pop("idx9"))
                    for t in gm1:
                        st_build(t)
                    QUS = 1.5              # PE us per combine quarter
                    LAT = 11.0             # per-level ladder chain latency
                    STAG = 3.0             # per-tile dot stagger on DVE
                    sched = []
                    for lv_i, d in enumerate(DEEP_LV[1:], start=1):
                        for t_i, t in enumerate(gm1):
                            sched.append(
                                (4.0 + (lv_i - 1) * LAT + t_i * STAG,
                                 d, t_i, t))
                    sched.sort()
                    pos = 0.0
                    for when, d, t_i, t in sched:
                        want = int(max(0.0, when - pos) / QUS + 0.5)
                        nq_emit = min(want, len(comb))
                        emit_comb(nq_emit)
                        pos += nq_emit * QUS
                        deep_consume(t, d - 1)
                        deep_issue(t, d)
                    emit_comb(len(comb))
                    for t in gm1:
                        deep_consume(t, LAST_LV)
                    for t in gm1:
                        deep_finish(t)
                else:
                    # classic: ladder fully inside this slot, level-major,
                    # combine chunks of group m-2 between levels
                    emit_comb(1)
                    for t in gm1:
                        deep_issue(t, SH_LV)
                    nlv = len(DEEP_LV)
                    for lv_i, d in enumerate(DEEP_LV):
                        if d > SH_LV:
                            left = nlv - lv_i + 1
                            emit_comb(max(1, len(comb) // left))
                            for t in gm1:
                                deep_issue(t, d)
                        for t in gm1:
                            deep_consume(t, d)
                    emit_comb(len(comb))
                    for t in gm1:
                        deep_finish(t)
                    for t in gm1:
                        st_build(t)
                for t in gm2:
                    state.pop(t, None)
                # next group's transposes + logits + its shallow routing,
                # so the next slot's ladder starts without routing latency
                for t in gm0:
                    s0_transpose(t)
                    s0_logits(t)
                for t in gm0:
                    s1_shallow(t)
                if OPTS["early9"]:
                    for t in gm0:
                        deep_issue(t, SH_LV)
                # x prefetch for group m+1
                for t in gp1:
                    s0_load(t)

            # prologue: x for group 0 interleaved with the shallow tables
            nw1t = OPTS.get("w1t_pieces", 4)
            s0_load(groups[0][0], eng=nc.sync)
            load_w1t_chunk(0, nw1t)
            for t in groups[0][1:]:
                s0_load(t, eng=nc.sync)
            for r in range(1, nw1t):
                load_w1t_chunk(r, nw1t)
            for r in range(2):
                load_w2sh_chunk(r)
            for m in range(ng + 2):
                emit_slot(m)

    nc.compile()
    return nc


_CACHED = {}


def _get_program(n_tiles=NT, num_devices=N_CORES):
    key = (n_tiles, num_devices)
    if key not in _CACHED:
        _CACHED[key] = build_program(n_tiles, num_devices)
    return _CACHED[key]


def idx_masks():
    i = np.arange(P)
    wsel = (i[:, None] % 16 == i[None, :] % 16).astype(np.float32)
    m8 = (i[:, None] // 16 == np.arange(8)[None, :]).astype(np.float32)
    return wsel, m8


def prep_inputs(input, w1s, w2s):
    """Host-side layout prep shared by all cores."""
    w1 = np.asarray(w1s)
    w1t_sh = np.zeros((IN, SH_PAD), dtype=w1.dtype)
    w1t_sh[:, :SH_NODES] = w1[:SH_NODES].T
    w1q8 = (w1.astype(np.float32) * W1SCALE).astype(ml_dtypes.float8_e3m4)
    w2q = (np.asarray(w2s).astype(np.float32) * W2SCALE).astype(
        ml_dtypes.float8_e3m4)
    return np.ascontiguousarray(w1t_sh), w1q8, w2q


def _run(input, w1s, w2s, **spmd_kwargs):
    from concourse.bass_utils import run_bass_kernel_spmd

    nc = _get_program()
    w1t_sh, w1q8, w2q = prep_inputs(input, w1s, w2s)
    w1 = np.ascontiguousarray(np.asarray(w1s))
    wsel, m8 = idx_masks()
    in_maps = []
    for c in range(N_CORES):
        im = {
            "x": np.ascontiguousarray(np.asarray(input)[c * TOK:(c + 1) * TOK]),
            "w1t_sh": w1t_sh,
            "w1s": w1,
            "w1q8": w1q8,
            "w2q": w2q,
            "wsel": wsel,
            "m8": m8,
        }
        in_maps.append(im)
    res = run_bass_kernel_spmd(nc, in_maps, core_ids=list(range(N_CORES)),
                               **spmd_kwargs)
    out = np.concatenate([res.results[c]["y"] for c in range(N_CORES)],
                         axis=0)
    return out.astype(ml_dtypes.bfloat16), res


def kernel(input, w1s, w2s, depth):
    assert int(depth) == DEPTH
    out, _ = _run(input, w1s, w2s)
    return out
